# revision 2
# baseline (speedup 1.0000x reference)
"""Fully-fused single-launch TRN2 kernel for nn_Decoder_1700807049879.

Per core (1 sample), everything on device except the tiny routing MLP
(host, f64) whose softmax weights are folded into the uploaded spectral
filter G:

  S1  pw1 + relu^2    -> x_pre [w, h*192+m]           (fp16 SBUF)
  S2  rfft-W (packed) -> XwT [kwf, m*128+h]; xbar T -> Xw [h, m*128+kwf]
  S3/M/S5 interleaved per 16-channel block:
      DFT-H -> Xf_blk; Y_blk = Xf_blk*G_blk; iDFT-H -> Zh [h, m*128+fold]
  T1  xbar transpose  -> Zfold [kwf, m*128+h]
  S6  irfft-W folded  -> xspA [w, h*128+m(0:128)], xspB [w, h*128+(m-96)]
  T2  xbar transposes -> dstA [m, h*128+w], dstB
  S7  pw2             -> out [c, h*128+w] -> DRAM

rfft folding: for real rows Im(kw=0)=Im(kw=64)=0, so the W-stage packs
[Cw(0:65) | -Sw(1:64)] into one 128-wide stationary and the inverse
packs [alpha*cos ; -alpha*sin] into one 128-deep contraction.
"""

import os
import sys
import numpy as np
from contextlib import ExitStack

sys.path.insert(0, "/opt/trn_rl_repo")

from concourse import bass, bacc, mybir, tile  # noqa: E402
from concourse.bass_utils import run_bass_kernel_spmd  # noqa: E402

B, H, W, DIM = 8, 128, 128, 96
MED, NS, SCTX = 192, 3, 48
FH, FWH = 128, 65
SCALE_HW = [(16, 9), (8, 4), (24, 13)]
S = H * W
F16 = mybir.dt.float16
F32 = mybir.dt.float32

MBLK = 12              # channels per S3/M/S5 block (== transpose chunk)
NBLK = MED // MBLK     # 16
FDB = MBLK * 65        # 780
GCH = 6                # channels per H-DFT psum group (N=390 <= 512)
CCH = 12               # channels per transpose chunk (1 block)


# ---------------------------------------------------------------- host math
def _cubic(t, a=-0.75):
    t = abs(t)
    if t <= 1.0:
        return (a + 2) * t ** 3 - (a + 3) * t ** 2 + 1.0
    if t < 2.0:
        return a * t ** 3 - 5 * a * t ** 2 + 8 * a * t - 4 * a
    return 0.0


def _resize_mat(old, new):
    M = np.zeros((new, old), dtype=np.float64)
    for j in range(new):
        s = j * (old - 1) / (new - 1) if new > 1 else 0.0
        f = int(np.floor(s))
        for k in range(-1, 3):
            M[j, min(max(f + k, 0), old - 1)] += _cubic(s - (f + k))
    return M


def _dft_consts():
    k = np.arange(128)
    ang = 2 * np.pi * np.outer(k, k) / 128.0
    C = np.cos(ang) / np.sqrt(128.0)
    Sm = np.sin(ang) / np.sqrt(128.0)
    CWF = np.concatenate([C[:, 0:65], -Sm[:, 1:64]], axis=1)
    alpha = np.ones(65)
    alpha[1:64] = 2.0
    cwa = (alpha[:, None]
           * np.cos(2 * np.pi * np.outer(np.arange(65), k) / 128.0)
           / np.sqrt(128.0))
    swa = (2.0 * np.sin(2 * np.pi * np.outer(np.arange(1, 64), k) / 128.0)
           / np.sqrt(128.0))
    IWF = np.concatenate([cwa, -swa], axis=0)
    return (CWF.astype(np.float16), C.astype(np.float16),
            Sm.astype(np.float16), (-Sm).astype(np.float16),
            IWF.astype(np.float16))


def _host_routing_G(x, cw0, cw1, cw2, sp_w, bn_gamma, bn_beta, bn_mean,
                    bn_var, fc1, mlp_scale, mlp_bias, fc2, soa1_scale):
    x = np.asarray(x, np.float64)
    gctx = x.mean(axis=(1, 2))
    y = np.einsum('bhwc,sc->bhws', x, np.asarray(sp_w, np.float64))
    y = ((y - np.asarray(bn_mean, np.float64))
         / np.sqrt(np.asarray(bn_var, np.float64) + 1e-5)
         * np.asarray(bn_gamma, np.float64) + np.asarray(bn_beta, np.float64))
    sctx = np.maximum(y, 0.0).mean(axis=(1, 2))
    fused = np.concatenate([gctx, sctx], axis=1)
    hm = fused @ np.asarray(fc1, np.float64).T
    ms = float(np.asarray(mlp_scale).reshape(-1)[0])
    mb = float(np.asarray(mlp_bias).reshape(-1)[0])
    hmid = ms * np.maximum(hm, 0.0) ** 2 + mb
    logits = (hmid @ np.asarray(fc2, np.float64).T).reshape(B, NS, MED)
    e = np.exp(logits - logits.max(axis=1, keepdims=True))
    r = e / e.sum(axis=1, keepdims=True)
    filts = []
    for cw, (sh, sw) in zip((cw0, cw1, cw2), SCALE_HW):
        cw = np.asarray(cw, np.float64)
        Rh = _resize_mat(sh, FH).astype(np.float32)
        Rw = _resize_mat(sw, FWH).astype(np.float32)
        t = np.einsum('Vw,hwmc->hVmc', Rw, cw.astype(np.float32))
        t = np.einsum('Hh,hVmc->HVmc', Rh, t)
        filts.append((t[..., 0] + 1j * t[..., 1]).astype(np.complex64))
    filt = np.stack(filts)                                  # [3, kh, kw, m]
    sc = float(np.asarray(soa1_scale).reshape(-1)[0])
    G = np.einsum('skwm,bsm->bkmw', filt,
                  r.astype(np.complex64)) * sc              # [B, kh, m, kw]
    return r, G


# ---------------------------------------------------------------- device
def _build():
    nc = bacc.Bacc("TRN2", target_bir_lowering=False, debug=False,
                   num_devices=B)
    xt = nc.dram_tensor("xt", [DIM, S], F16, kind="ExternalInput").ap()
    gg = nc.dram_tensor("gg", [128, 2 * 192 * 65], F16, kind="ExternalInput").ap()
    w1x_d = nc.dram_tensor("w1x", [96, 192], F16, kind="ExternalInput").ap()
    cwf_d = nc.dram_tensor("cwf", [128, 128], F16, kind="ExternalInput").ap()
    cht_d = nc.dram_tensor("cht", [128, 128], F16, kind="ExternalInput").ap()
    sht_d = nc.dram_tensor("sht", [128, 128], F16, kind="ExternalInput").ap()
    nsht_d = nc.dram_tensor("nsht", [128, 128], F16, kind="ExternalInput").ap()
    iwf_d = nc.dram_tensor("iwf", [128, 128], F16, kind="ExternalInput").ap()
    w2a_d = nc.dram_tensor("w2a", [96, 96], F16, kind="ExternalInput").ap()
    w2b_d = nc.dram_tensor("w2b", [96, 96], F16, kind="ExternalInput").ap()
    out_d = nc.dram_tensor("out", [DIM, S], F16, kind="ExternalOutput").ap()

    with tile.TileContext(nc) as tc, ExitStack() as ctx:
        cpool = ctx.enter_context(tc.tile_pool(name="c", bufs=1))
        big = ctx.enter_context(tc.tile_pool(name="big", bufs=2))
        chp = ctx.enter_context(tc.tile_pool(name="ch", bufs=2))
        spc = ctx.enter_context(tc.tile_pool(name="sc", bufs=3))
        xpool = ctx.enter_context(tc.tile_pool(name="xp", bufs=2))
        fpool = ctx.enter_context(tc.tile_pool(name="fp", bufs=3))
        gpool = ctx.enter_context(tc.tile_pool(name="gp", bufs=2))
        tpool = ctx.enter_context(tc.tile_pool(name="tp", bufs=2))
        rpool = ctx.enter_context(tc.tile_pool(name="rp", bufs=3))
        opool = ctx.enter_context(tc.tile_pool(name="op", bufs=2))
        pp = ctx.enter_context(tc.tile_pool(name="ps", bufs=6, space="PSUM"))
        pp7 = ctx.enter_context(tc.tile_pool(name="p7", bufs=2, space="PSUM"))

        w1x = cpool.tile([96, 192], F16)
        cwf = cpool.tile([128, 128], F16, tag="cwf")
        cht = cpool.tile([128, 128], F16, tag="cht")
        sht = cpool.tile([128, 128], F16, tag="sht")
        nsht = cpool.tile([128, 128], F16, tag="nsht")
        iwf = cpool.tile([128, 128], F16, tag="iwf")
        w2a = cpool.tile([96, 96], F16, tag="w2a")
        w2b = cpool.tile([96, 96], F16, tag="w2b")
        nc.sync.dma_start(w1x[:], w1x_d[:])
        nc.sync.dma_start(cwf[:], cwf_d[:])
        nc.sync.dma_start(cht[:], cht_d[:])
        nc.sync.dma_start(sht[:], sht_d[:])
        nc.sync.dma_start(nsht[:], nsht_d[:])
        nc.sync.dma_start(iwf[:], iwf_d[:])
        nc.sync.dma_start(w2a[:], w2a_d[:])
        nc.sync.dma_start(w2b[:], w2b_d[:])

        BW = 24576          # big tile free width
        SCH = 48            # channels per S2 chunk
        SCW = SCH * 128     # 6144
        TCH = 24            # channels per T1 chunk (2 blocks)
        TCW = TCH * 128     # 3072

        # ---------------- S1: pw1 + relu^2 -> x_pre [w, h*192+m]
        x_pre = big.tile([128, BW], F16, tag="big")
        XCH = 1024  # xt chunk cols (8 h)
        for ci in range(S // XCH):
            xc = xpool.tile([96, XCH], F16, tag="xt")
            nc.sync.dma_start(xc[:], xt[:, ci * XCH:(ci + 1) * XCH])
            for hh in range(0, 8, 2):
                h = ci * 8 + hh
                ps = pp.tile([128, 512], F32, tag="ps")
                for j in range(2):
                    nc.tensor.matmul(
                        ps[:, j * 192:(j + 1) * 192],
                        xc[:, (hh + j) * 128:(hh + j + 1) * 128],
                        w1x[:], start=True, stop=True)
                # relu^2: alternate (ACT relu, DVE sq) / (DVE max0, ACT sq)
                rt = rpool.tile([128, 384], F16, tag="rt")
                if (h // 2) % 2 == 0:
                    nc.scalar.activation(rt[:], ps[:, 0:384],
                                         mybir.ActivationFunctionType.Relu)
                    nc.vector.tensor_mul(x_pre[:, h * 192:(h + 2) * 192],
                                         rt[:], rt[:])
                else:
                    nc.vector.tensor_scalar_max(rt[:], ps[:, 0:384], 0.0)
                    nc.scalar.square(x_pre[:, h * 192:(h + 2) * 192], rt[:])

        # ---------------- S2: rfft-W packed, chunked (48 ch) -> Xw chunks
        xp3 = x_pre[:].rearrange("p (h m) -> p m h", h=128, m=192)
        xw_chunks = {}

        def s2_chunk(c):
            xwt = chp.tile([128, SCW], F16, tag="xwt")
            for jj in range(SCH // 4):            # 12 matmuls of N=512
                j = c * (SCH // 4) + jj
                ps = pp.tile([128, 512], F32, tag="ps")
                nc.tensor.matmul(ps[:], cwf[:],
                                 xp3[:, j * 4:(j + 1) * 4, :],
                                 start=True, stop=True)
                if j % 2 == 0:
                    nc.scalar.copy(xwt[:, jj * 512:(jj + 1) * 512], ps[:])
                else:
                    nc.vector.tensor_copy(xwt[:, jj * 512:(jj + 1) * 512],
                                          ps[:])
            xw = chp.tile([128, SCW], F16, tag="xw")
            nc.sync.dma_start_transpose(
                xw[:].rearrange("p (j k) -> p j k", j=SCH, k=128), xwt[:])
            xw_chunks[c] = xw

        # ---------------- S3/M/S5 software-pipelined over 12-channel blocks
        Zf = big.tile([128, BW], F16, tag="big")
        zf3d = Zf[:].rearrange("p (j k) -> p j k", j=192, k=128)
        yts = {}
        zhs = {}

        def s3_and_m(blk):
            m0 = blk * MBLK
            xw3 = xw_chunks[blk // 4][:].rearrange(
                "p (m c) -> p m c", m=SCH, c=128)
            lm0 = (blk % 4) * MBLK
            gt = gpool.tile([128, 2 * FDB], F16, tag="g")
            nc.gpsimd.dma_start(gt[:], gg[:, blk * 2 * FDB:(blk + 1) * 2 * FDB])
            xf = fpool.tile([128, 2 * FDB], F16, tag="xf")
            for q in range(MBLK // GCH):
                g0 = lm0 + q * GCH
                NW = GCH * 65                     # 390
                re_in = xw3[:, g0:g0 + GCH, 0:65]
                im_in = xw3[:, g0:g0 + GCH, 65:128]
                psr = pp.tile([128, 512], F32, tag="ps")
                psi = pp.tile([128, 512], F32, tag="ps")
                psr3 = psr[:, 0:NW].rearrange("p (m c) -> p m c", m=GCH, c=65)
                psi3 = psi[:, 0:NW].rearrange("p (m c) -> p m c", m=GCH, c=65)
                nc.tensor.matmul(psr[:, 0:NW], cht[:], re_in,
                                 start=True, stop=False)
                nc.tensor.matmul(psr3[:, :, 1:64], sht[:], im_in,
                                 start=False, stop=True)
                nc.tensor.matmul(psi[:, 0:NW], nsht[:], re_in,
                                 start=True, stop=False)
                nc.tensor.matmul(psi3[:, :, 1:64], cht[:], im_in,
                                 start=False, stop=True)
                o = q * NW
                nc.scalar.copy(xf[:, o:o + NW], psr[:, 0:NW])
                nc.vector.tensor_copy(xf[:, FDB + o:FDB + o + NW],
                                      psi[:, 0:NW])
            # M: Y = Xf * G (1 mul POOL, 3 muls + combines DVE)
            xfre, xfim = xf[:, 0:FDB], xf[:, FDB:2 * FDB]
            gre_t, gim_t = gt[:, 0:FDB], gt[:, FDB:2 * FDB]
            yt = fpool.tile([128, 2 * FDB], F16, tag="y")
            t1 = tpool.tile([128, FDB], F16, tag="mt")
            t2 = tpool.tile([128, FDB], F16, tag="mt")
            nc.gpsimd.tensor_mul(t1[:], xfre[:], gre_t[:])
            nc.vector.tensor_mul(t2[:], xfim[:], gim_t[:])
            nc.vector.tensor_sub(yt[:, 0:FDB], t1[:], t2[:])
            t3 = tpool.tile([128, FDB], F16, tag="mt")
            t4 = tpool.tile([128, FDB], F16, tag="mt")
            nc.vector.tensor_mul(t3[:], xfre[:], gim_t[:])
            nc.vector.tensor_mul(t4[:], xfim[:], gre_t[:])
            nc.vector.tensor_add(yt[:, FDB:2 * FDB], t3[:], t4[:])
            yts[blk] = yt

        def s5(blk):
            if blk % 2 == 0:
                zhc = chp.tile([128, TCW], F16, tag="zh")
                zhs[blk // 2] = zhc
            zh = zhs[blk // 2]
            lm0 = (blk % 2) * MBLK
            yt = yts.pop(blk)
            yre3 = yt[:, 0:FDB].rearrange("p (m c) -> p m c", m=MBLK, c=65)
            yim3 = yt[:, FDB:2 * FDB].rearrange("p (m c) -> p m c",
                                                m=MBLK, c=65)
            for q in range(MBLK // GCH):
                q0 = q * GCH
                NW = GCH * 65                     # 390
                NI = GCH * 63                     # 378
                re_in = yre3[:, q0:q0 + GCH, :]
                im_in = yim3[:, q0:q0 + GCH, :]
                psr = pp.tile([128, 512], F32, tag="ps")
                psi = pp.tile([128, 512], F32, tag="ps")
                nc.tensor.matmul(psr[:, 0:NW], cht[:], re_in,
                                 start=True, stop=False)
                nc.tensor.matmul(psr[:, 0:NW], nsht[:], im_in,
                                 start=False, stop=True)
                nc.tensor.matmul(psi[:, 0:NI], sht[:],
                                 yre3[:, q0:q0 + GCH, 1:64],
                                 start=True, stop=False)
                nc.tensor.matmul(psi[:, 0:NI], cht[:],
                                 yim3[:, q0:q0 + GCH, 1:64],
                                 start=False, stop=True)
                zh3 = zh[:, (lm0 + q0) * 128:(lm0 + q0 + GCH) * 128].rearrange(
                    "p (m c) -> p m c", m=GCH, c=128)
                psr3 = psr[:, 0:NW].rearrange("p (m c) -> p m c", m=GCH, c=65)
                psi3 = psi[:, 0:NI].rearrange("p (m c) -> p m c", m=GCH, c=63)
                nc.scalar.copy(zh3[:, :, 0:65], psr3[:])
                nc.scalar.copy(zh3[:, :, 65:128], psi3[:])
            if blk % 2 == 1:        # 24 channels done -> T1 chunk
                c = blk // 2
                nc.sync.dma_start_transpose(
                    zf3d[:, c * TCH:(c + 1) * TCH, :], zhs.pop(c)[:])

        s2_chunk(0)
        s2_chunk(1)
        for blk in range(NBLK):
            if blk % 4 == 0 and blk // 4 + 2 < MED // SCH:
                s2_chunk(blk // 4 + 2)
            s3_and_m(blk)
            if blk >= 2:
                s5(blk - 2)
        s5(NBLK - 2)
        s5(NBLK - 1)

        # ---------------- S6 (stationary-data irfft-W, no transpose) + S7
        # per 4-h group: lhsT = Zf[:, m*128+h] slices -> psum [m-chunk, 4*128]
        def s6_s7(hb):
            h4 = hb * 4
            dd = []
            for base in (0, 12288):              # m 0..95 | m 96..191
                ps = pp.tile([128, 512], F32, tag="ps")
                for k in range(4):
                    h = h4 + k
                    nc.tensor.matmul(
                        ps[0:96, k * 128:(k + 1) * 128],
                        Zf[:, base + h:base + h + 95 * 128 + 1:128],
                        iwf[:], start=True, stop=True)
                d = spc.tile([96, 512], F16, tag="dA" if base == 0 else "dB")
                if hb % 2 == 0:
                    nc.scalar.copy(d[:], ps[0:96, :])
                else:
                    nc.vector.tensor_copy(d[:], ps[0:96, :])
                dd.append(d)
            ps7 = pp7.tile([128, 512], F32, tag="p7")
            nc.tensor.matmul(ps7[0:96, :], w2a[:], dd[0][:],
                             start=True, stop=False)
            nc.tensor.matmul(ps7[0:96, :], w2b[:], dd[1][:],
                             start=False, stop=True)
            if hb % 4 == 0:
                obt = opool.tile([96, 2048], F16, tag="ob")
                obs[0] = obt
            ob = obs[0]
            sl = slice((hb % 4) * 512, (hb % 4 + 1) * 512)
            if hb % 2 == 0:
                nc.vector.tensor_copy(ob[:, sl], ps7[0:96, :])
            else:
                nc.scalar.copy(ob[:, sl], ps7[0:96, :])
            if hb % 4 == 3:
                nc.scalar.dma_start(
                    out_d[:, (h4 - 12) * 128:(h4 + 4) * 128], ob[:])

        obs = [None]
        for hb in range(32):
            s6_s7(hb)
    nc.finalize()
    return nc


_CACHE = {}
LAST_EXEC_NS = 0


def _consts_f16(w1, w2):
    CWF, CHT, SHT, NSHT, IWF = _dft_consts()
    return dict(
        w1x=np.ascontiguousarray(np.asarray(w1, np.float32).T).astype(np.float16),
        cwf=CWF, cht=CHT, sht=SHT, nsht=NSHT, iwf=IWF,
        w2a=np.ascontiguousarray(np.asarray(w2, np.float32).T[0:96]).astype(np.float16),
        w2b=np.ascontiguousarray(np.asarray(w2, np.float32).T[96:192]).astype(np.float16),
    )


def kernel(x, w1, soa1_scale, soa1_bias, cw0, cw1, cw2, sp_w,
           bn_gamma, bn_beta, bn_mean, bn_var,
           fc1, mlp_scale, mlp_bias, fc2, w2):
    global LAST_EXEC_NS
    x = np.asarray(x, np.float32)
    r, G = _host_routing_G(x, cw0, cw1, cw2, sp_w, bn_gamma, bn_beta,
                           bn_mean, bn_var, fc1, mlp_scale, mlp_bias, fc2,
                           soa1_scale)
    consts = _consts_f16(w1, w2)
    in_maps = []
    for b in range(B):
        xT = np.ascontiguousarray(x[b].reshape(S, DIM).T).astype(np.float16)
        m = dict(consts)
        m["xt"] = xT
        gr = G[b].real.reshape(128, NBLK, FDB)
        gi = G[b].imag.reshape(128, NBLK, FDB)
        ggb = np.empty((128, NBLK, 2 * FDB), np.float16)
        ggb[:, :, 0:FDB] = gr
        ggb[:, :, FDB:2 * FDB] = gi
        m["gg"] = np.ascontiguousarray(ggb.reshape(128, 2 * 192 * 65))
        in_maps.append(m)

    if "nc" not in _CACHE:
        _CACHE["nc"] = _build()
    nc = _CACHE["nc"]
    res = run_bass_kernel_spmd(nc, in_maps, list(range(B)))
    if os.environ.get("KERNEL_TRACE_EXEC") and "tl" not in _CACHE:
        from concourse.timeline_sim import TimelineSim
        _CACHE["tl"] = TimelineSim(nc, trace=False).simulate()
    if _CACHE.get("tl"):
        LAST_EXEC_NS = int(_CACHE["tl"])

    outs = np.empty((B, H, W, DIM), np.float32)
    for b in range(B):
        o = res.results[b]["out"].astype(np.float32)      # [c, hw]
        outs[b] = o.T.reshape(H, W, DIM)

    bias = float(np.asarray(soa1_bias).reshape(-1)[0])
    if bias != 0.0:
        comb00 = G[:, 0, :, 0]                            # [B, m]
        corr = np.real(comb00).astype(np.float64) @ np.asarray(w2, np.float64).T
        outs = outs + bias * corr[:, None, None, :].astype(np.float32)
    return outs


# revision 3
# speedup vs baseline: 1.0584x; 1.0584x over previous
"""Fully-fused single-launch TRN2 kernel for nn_Decoder_1700807049879.

Per core (1 sample), everything on device except the tiny routing MLP
(host, f64) whose softmax weights are folded into the uploaded spectral
filter G:

  S1  pw1 + relu^2    -> x_pre [w, h*192+m]           (fp16 SBUF)
  S2  rfft-W (packed) -> XwT [kwf, m*128+h]; xbar T -> Xw [h, m*128+kwf]
  S3/M/S5 interleaved per 16-channel block:
      DFT-H -> Xf_blk; Y_blk = Xf_blk*G_blk; iDFT-H -> Zh [h, m*128+fold]
  T1  xbar transpose  -> Zfold [kwf, m*128+h]
  S6  irfft-W folded  -> xspA [w, h*128+m(0:128)], xspB [w, h*128+(m-96)]
  T2  xbar transposes -> dstA [m, h*128+w], dstB
  S7  pw2             -> out [c, h*128+w] -> DRAM

rfft folding: for real rows Im(kw=0)=Im(kw=64)=0, so the W-stage packs
[Cw(0:65) | -Sw(1:64)] into one 128-wide stationary and the inverse
packs [alpha*cos ; -alpha*sin] into one 128-deep contraction.
"""

import os
import sys
import numpy as np
from contextlib import ExitStack

sys.path.insert(0, "/opt/trn_rl_repo")

from concourse import bass, bacc, mybir, tile  # noqa: E402
from concourse.bass_utils import run_bass_kernel_spmd  # noqa: E402

B, H, W, DIM = 8, 128, 128, 96
MED, NS, SCTX = 192, 3, 48
FH, FWH = 128, 65
SCALE_HW = [(16, 9), (8, 4), (24, 13)]
S = H * W
F16 = mybir.dt.float16
F32 = mybir.dt.float32

MBLK = 12              # channels per S3/M/S5 block (== transpose chunk)
NBLK = MED // MBLK     # 16
FDB = MBLK * 65        # 780
GCH = 6                # channels per H-DFT psum group (N=390 <= 512)
CCH = 12               # channels per transpose chunk (1 block)


# ---------------------------------------------------------------- host math
def _cubic(t, a=-0.75):
    t = abs(t)
    if t <= 1.0:
        return (a + 2) * t ** 3 - (a + 3) * t ** 2 + 1.0
    if t < 2.0:
        return a * t ** 3 - 5 * a * t ** 2 + 8 * a * t - 4 * a
    return 0.0


def _resize_mat(old, new):
    M = np.zeros((new, old), dtype=np.float64)
    for j in range(new):
        s = j * (old - 1) / (new - 1) if new > 1 else 0.0
        f = int(np.floor(s))
        for k in range(-1, 3):
            M[j, min(max(f + k, 0), old - 1)] += _cubic(s - (f + k))
    return M


def _dft_consts():
    k = np.arange(128)
    ang = 2 * np.pi * np.outer(k, k) / 128.0
    C = np.cos(ang) / np.sqrt(128.0)
    Sm = np.sin(ang) / np.sqrt(128.0)
    CWF = np.concatenate([C[:, 0:65], -Sm[:, 1:64]], axis=1)
    alpha = np.ones(65)
    alpha[1:64] = 2.0
    cwa = (alpha[:, None]
           * np.cos(2 * np.pi * np.outer(np.arange(65), k) / 128.0)
           / np.sqrt(128.0))
    swa = (2.0 * np.sin(2 * np.pi * np.outer(np.arange(1, 64), k) / 128.0)
           / np.sqrt(128.0))
    IWF = np.concatenate([cwa, -swa], axis=0)
    return (CWF.astype(np.float16), C.astype(np.float16),
            Sm.astype(np.float16), (-Sm).astype(np.float16),
            IWF.astype(np.float16))


def _host_routing_G(x, cw0, cw1, cw2, sp_w, bn_gamma, bn_beta, bn_mean,
                    bn_var, fc1, mlp_scale, mlp_bias, fc2, soa1_scale):
    x = np.asarray(x, np.float64)
    gctx = x.mean(axis=(1, 2))
    y = np.einsum('bhwc,sc->bhws', x, np.asarray(sp_w, np.float64))
    y = ((y - np.asarray(bn_mean, np.float64))
         / np.sqrt(np.asarray(bn_var, np.float64) + 1e-5)
         * np.asarray(bn_gamma, np.float64) + np.asarray(bn_beta, np.float64))
    sctx = np.maximum(y, 0.0).mean(axis=(1, 2))
    fused = np.concatenate([gctx, sctx], axis=1)
    hm = fused @ np.asarray(fc1, np.float64).T
    ms = float(np.asarray(mlp_scale).reshape(-1)[0])
    mb = float(np.asarray(mlp_bias).reshape(-1)[0])
    hmid = ms * np.maximum(hm, 0.0) ** 2 + mb
    logits = (hmid @ np.asarray(fc2, np.float64).T).reshape(B, NS, MED)
    e = np.exp(logits - logits.max(axis=1, keepdims=True))
    r = e / e.sum(axis=1, keepdims=True)
    filts = []
    for cw, (sh, sw) in zip((cw0, cw1, cw2), SCALE_HW):
        cw = np.asarray(cw, np.float64)
        Rh = _resize_mat(sh, FH).astype(np.float32)
        Rw = _resize_mat(sw, FWH).astype(np.float32)
        t = np.einsum('Vw,hwmc->hVmc', Rw, cw.astype(np.float32))
        t = np.einsum('Hh,hVmc->HVmc', Rh, t)
        filts.append((t[..., 0] + 1j * t[..., 1]).astype(np.complex64))
    filt = np.stack(filts)                                  # [3, kh, kw, m]
    sc = float(np.asarray(soa1_scale).reshape(-1)[0])
    G = np.einsum('skwm,bsm->bkmw', filt,
                  r.astype(np.complex64)) * sc              # [B, kh, m, kw]
    return r, G


# ---------------------------------------------------------------- device
def _build():
    nc = bacc.Bacc("TRN2", target_bir_lowering=False, debug=False,
                   num_devices=B)
    xt = nc.dram_tensor("xt", [DIM, S], F16, kind="ExternalInput").ap()
    gg = nc.dram_tensor("gg", [128, 2 * 192 * 65], F16, kind="ExternalInput").ap()
    w1x_d = nc.dram_tensor("w1x", [96, 192], F16, kind="ExternalInput").ap()
    cwf_d = nc.dram_tensor("cwf", [128, 128], F16, kind="ExternalInput").ap()
    cht_d = nc.dram_tensor("cht", [128, 128], F16, kind="ExternalInput").ap()
    sht_d = nc.dram_tensor("sht", [128, 128], F16, kind="ExternalInput").ap()
    nsht_d = nc.dram_tensor("nsht", [128, 128], F16, kind="ExternalInput").ap()
    iwf_d = nc.dram_tensor("iwf", [128, 128], F16, kind="ExternalInput").ap()
    w2a_d = nc.dram_tensor("w2a", [96, 96], F16, kind="ExternalInput").ap()
    w2b_d = nc.dram_tensor("w2b", [96, 96], F16, kind="ExternalInput").ap()
    out_d = nc.dram_tensor("out", [DIM, S], F16, kind="ExternalOutput").ap()

    with tile.TileContext(nc) as tc, ExitStack() as ctx:
        cpool = ctx.enter_context(tc.tile_pool(name="c", bufs=1))
        big = ctx.enter_context(tc.tile_pool(name="big", bufs=2))
        chp = ctx.enter_context(tc.tile_pool(name="ch", bufs=3))
        spc = ctx.enter_context(tc.tile_pool(name="sc", bufs=3))
        xpool = ctx.enter_context(tc.tile_pool(name="xp", bufs=2))
        fpool = ctx.enter_context(tc.tile_pool(name="fp", bufs=3))
        gpool = ctx.enter_context(tc.tile_pool(name="gp", bufs=4))
        tpool = ctx.enter_context(tc.tile_pool(name="tp", bufs=2))
        rpool = ctx.enter_context(tc.tile_pool(name="rp", bufs=3))
        opool = ctx.enter_context(tc.tile_pool(name="op", bufs=2))
        pp = ctx.enter_context(tc.tile_pool(name="ps", bufs=6, space="PSUM"))
        pp7 = ctx.enter_context(tc.tile_pool(name="p7", bufs=2, space="PSUM"))

        w1x = cpool.tile([96, 192], F16)
        cwf = cpool.tile([128, 128], F16, tag="cwf")
        cht = cpool.tile([128, 128], F16, tag="cht")
        sht = cpool.tile([128, 128], F16, tag="sht")
        nsht = cpool.tile([128, 128], F16, tag="nsht")
        iwf = cpool.tile([128, 128], F16, tag="iwf")
        w2a = cpool.tile([96, 96], F16, tag="w2a")
        w2b = cpool.tile([96, 96], F16, tag="w2b")
        nc.sync.dma_start(w1x[:], w1x_d[:])
        nc.sync.dma_start(cwf[:], cwf_d[:])
        nc.sync.dma_start(cht[:], cht_d[:])
        nc.sync.dma_start(sht[:], sht_d[:])
        nc.sync.dma_start(nsht[:], nsht_d[:])
        nc.sync.dma_start(iwf[:], iwf_d[:])
        nc.sync.dma_start(w2a[:], w2a_d[:])
        nc.sync.dma_start(w2b[:], w2b_d[:])

        BW = 24576          # big tile free width
        SCH = 24            # channels per S2 chunk (2 blocks)
        SCW = SCH * 128     # 3072
        TCH = 24            # channels per T1 chunk (2 blocks)
        TCW = TCH * 128     # 3072

        # ---------------- S1: pw1 + relu^2 -> x_pre [w, h*192+m]
        x_pre = big.tile([128, BW], F16, tag="big")
        XCH = 1024  # xt chunk cols (8 h)
        for ci in range(S // XCH):
            xc = xpool.tile([96, XCH], F16, tag="xt")
            nc.sync.dma_start(xc[:], xt[:, ci * XCH:(ci + 1) * XCH])
            for hh in range(0, 8, 2):
                h = ci * 8 + hh
                ps = pp.tile([128, 512], F32, tag="ps")
                for j in range(2):
                    nc.tensor.matmul(
                        ps[:, j * 192:(j + 1) * 192],
                        xc[:, (hh + j) * 128:(hh + j + 1) * 128],
                        w1x[:], start=True, stop=True)
                # relu^2: alternate (ACT relu, DVE sq) / (DVE max0, ACT sq)
                rt = rpool.tile([128, 384], F16, tag="rt")
                if (h // 2) % 2 == 0:
                    nc.scalar.activation(rt[:], ps[:, 0:384],
                                         mybir.ActivationFunctionType.Relu)
                    nc.vector.tensor_mul(x_pre[:, h * 192:(h + 2) * 192],
                                         rt[:], rt[:])
                else:
                    nc.vector.tensor_scalar_max(rt[:], ps[:, 0:384], 0.0)
                    nc.scalar.square(x_pre[:, h * 192:(h + 2) * 192], rt[:])

        # ---------------- S2: rfft-W packed, chunked (48 ch) -> Xw chunks
        xp3 = x_pre[:].rearrange("p (h m) -> p m h", h=128, m=192)
        xw_chunks = {}

        def s2_chunk(c):
            xwt = chp.tile([128, SCW], F16, tag="xwt")
            for jj in range(SCH // 4):            # 6 matmuls of N=512
                j = c * (SCH // 4) + jj
                ps = pp.tile([128, 512], F32, tag="ps")
                nc.tensor.matmul(ps[:], cwf[:],
                                 xp3[:, j * 4:(j + 1) * 4, :],
                                 start=True, stop=True)
                if j % 2 == 0:
                    nc.scalar.copy(xwt[:, jj * 512:(jj + 1) * 512], ps[:])
                else:
                    nc.vector.tensor_copy(xwt[:, jj * 512:(jj + 1) * 512],
                                          ps[:])
            xw = chp.tile([128, SCW], F16, tag="xw")
            nc.sync.dma_start_transpose(
                xw[:].rearrange("p (j k) -> p j k", j=SCH, k=128), xwt[:])
            xw_chunks[c] = xw

        # ---------------- S3/M/S5 software-pipelined over 12-channel blocks
        Zf = big.tile([128, BW], F16, tag="big")
        zf3d = Zf[:].rearrange("p (j k) -> p j k", j=192, k=128)
        yts = {}
        zhs = {}

        def s3_and_m(blk):
            m0 = blk * MBLK
            xw3 = xw_chunks[blk // 2][:].rearrange(
                "p (m c) -> p m c", m=SCH, c=128)
            lm0 = (blk % 2) * MBLK
            gt = gpool.tile([128, 2 * FDB], F16, tag="g")
            nc.gpsimd.dma_start(gt[:], gg[:, blk * 2 * FDB:(blk + 1) * 2 * FDB])
            xf = fpool.tile([128, 2 * FDB], F16, tag="xf")
            for q in range(MBLK // GCH):
                g0 = lm0 + q * GCH
                NW = GCH * 65                     # 390
                re_in = xw3[:, g0:g0 + GCH, 0:65]
                im_in = xw3[:, g0:g0 + GCH, 65:128]
                psr = pp.tile([128, 512], F32, tag="ps")
                psi = pp.tile([128, 512], F32, tag="ps")
                psr3 = psr[:, 0:NW].rearrange("p (m c) -> p m c", m=GCH, c=65)
                psi3 = psi[:, 0:NW].rearrange("p (m c) -> p m c", m=GCH, c=65)
                nc.tensor.matmul(psr[:, 0:NW], cht[:], re_in,
                                 start=True, stop=False)
                nc.tensor.matmul(psr3[:, :, 1:64], sht[:], im_in,
                                 start=False, stop=True)
                nc.tensor.matmul(psi[:, 0:NW], nsht[:], re_in,
                                 start=True, stop=False)
                nc.tensor.matmul(psi3[:, :, 1:64], cht[:], im_in,
                                 start=False, stop=True)
                o = q * NW
                nc.scalar.copy(xf[:, o:o + NW], psr[:, 0:NW])
                nc.vector.tensor_copy(xf[:, FDB + o:FDB + o + NW],
                                      psi[:, 0:NW])
            # M: Y = Xf * G (1 mul POOL, 3 muls + combines DVE)
            xfre, xfim = xf[:, 0:FDB], xf[:, FDB:2 * FDB]
            gre_t, gim_t = gt[:, 0:FDB], gt[:, FDB:2 * FDB]
            yt = fpool.tile([128, 2 * FDB], F16, tag="y")
            t1 = tpool.tile([128, FDB], F16, tag="mt")
            t2 = tpool.tile([128, FDB], F16, tag="mt")
            nc.gpsimd.tensor_mul(t1[:], xfre[:], gre_t[:])
            nc.vector.tensor_mul(t2[:], xfim[:], gim_t[:])
            nc.vector.tensor_sub(yt[:, 0:FDB], t1[:], t2[:])
            t3 = tpool.tile([128, FDB], F16, tag="mt")
            t4 = tpool.tile([128, FDB], F16, tag="mt")
            nc.vector.tensor_mul(t3[:], xfre[:], gim_t[:])
            nc.vector.tensor_mul(t4[:], xfim[:], gre_t[:])
            nc.vector.tensor_add(yt[:, FDB:2 * FDB], t3[:], t4[:])
            yts[blk] = yt

        def s5(blk):
            if blk % 2 == 0:
                zhc = chp.tile([128, TCW], F16, tag="zh")
                zhs[blk // 2] = zhc
            zh = zhs[blk // 2]
            lm0 = (blk % 2) * MBLK
            yt = yts.pop(blk)
            yre3 = yt[:, 0:FDB].rearrange("p (m c) -> p m c", m=MBLK, c=65)
            yim3 = yt[:, FDB:2 * FDB].rearrange("p (m c) -> p m c",
                                                m=MBLK, c=65)
            for q in range(MBLK // GCH):
                q0 = q * GCH
                NW = GCH * 65                     # 390
                NI = GCH * 63                     # 378
                re_in = yre3[:, q0:q0 + GCH, :]
                im_in = yim3[:, q0:q0 + GCH, :]
                psr = pp.tile([128, 512], F32, tag="ps")
                psi = pp.tile([128, 512], F32, tag="ps")
                nc.tensor.matmul(psr[:, 0:NW], cht[:], re_in,
                                 start=True, stop=False)
                nc.tensor.matmul(psr[:, 0:NW], nsht[:], im_in,
                                 start=False, stop=True)
                nc.tensor.matmul(psi[:, 0:NI], sht[:],
                                 yre3[:, q0:q0 + GCH, 1:64],
                                 start=True, stop=False)
                nc.tensor.matmul(psi[:, 0:NI], cht[:],
                                 yim3[:, q0:q0 + GCH, 1:64],
                                 start=False, stop=True)
                zh3 = zh[:, (lm0 + q0) * 128:(lm0 + q0 + GCH) * 128].rearrange(
                    "p (m c) -> p m c", m=GCH, c=128)
                psr3 = psr[:, 0:NW].rearrange("p (m c) -> p m c", m=GCH, c=65)
                psi3 = psi[:, 0:NI].rearrange("p (m c) -> p m c", m=GCH, c=63)
                nc.scalar.copy(zh3[:, :, 0:65], psr3[:])
                nc.scalar.copy(zh3[:, :, 65:128], psi3[:])
            if blk % 2 == 1:        # 24 channels done -> T1 chunk
                c = blk // 2
                nc.sync.dma_start_transpose(
                    zf3d[:, c * TCH:(c + 1) * TCH, :], zhs.pop(c)[:])

        s2_chunk(0)
        s2_chunk(1)
        for blk in range(NBLK):
            if blk % 2 == 0 and blk // 2 + 2 < MED // SCH:
                s2_chunk(blk // 2 + 2)
            s3_and_m(blk)
            if blk >= 2:
                s5(blk - 2)
        s5(NBLK - 2)
        s5(NBLK - 1)

        # ---------------- S6 (stationary-data irfft-W, no transpose) + S7
        # per 4-h group: lhsT = Zf[:, m*128+h] slices -> psum [m-chunk, 4*128]
        def s6_s7(hb):
            h4 = hb * 4
            dd = []
            for base in (0, 12288):              # m 0..95 | m 96..191
                ps = pp.tile([128, 512], F32, tag="ps")
                for k in range(4):
                    h = h4 + k
                    nc.tensor.matmul(
                        ps[0:96, k * 128:(k + 1) * 128],
                        Zf[:, base + h:base + h + 95 * 128 + 1:128],
                        iwf[:], start=True, stop=True)
                d = spc.tile([96, 512], F16, tag="dA" if base == 0 else "dB")
                if hb % 2 == 0:
                    nc.scalar.copy(d[:], ps[0:96, :])
                else:
                    nc.vector.tensor_copy(d[:], ps[0:96, :])
                dd.append(d)
            ps7 = pp7.tile([128, 512], F32, tag="p7")
            nc.tensor.matmul(ps7[0:96, :], w2a[:], dd[0][:],
                             start=True, stop=False)
            nc.tensor.matmul(ps7[0:96, :], w2b[:], dd[1][:],
                             start=False, stop=True)
            if hb % 4 == 0:
                obt = opool.tile([96, 2048], F16, tag="ob")
                obs[0] = obt
            ob = obs[0]
            sl = slice((hb % 4) * 512, (hb % 4 + 1) * 512)
            if hb % 2 == 0:
                nc.vector.tensor_copy(ob[:, sl], ps7[0:96, :])
            else:
                nc.scalar.copy(ob[:, sl], ps7[0:96, :])
            if hb % 4 == 3:
                nc.scalar.dma_start(
                    out_d[:, (h4 - 12) * 128:(h4 + 4) * 128], ob[:])

        obs = [None]
        for hb in range(32):
            s6_s7(hb)
    nc.finalize()
    return nc


_CACHE = {}
LAST_EXEC_NS = 0


def _consts_f16(w1, w2):
    CWF, CHT, SHT, NSHT, IWF = _dft_consts()
    return dict(
        w1x=np.ascontiguousarray(np.asarray(w1, np.float32).T).astype(np.float16),
        cwf=CWF, cht=CHT, sht=SHT, nsht=NSHT, iwf=IWF,
        w2a=np.ascontiguousarray(np.asarray(w2, np.float32).T[0:96]).astype(np.float16),
        w2b=np.ascontiguousarray(np.asarray(w2, np.float32).T[96:192]).astype(np.float16),
    )


def kernel(x, w1, soa1_scale, soa1_bias, cw0, cw1, cw2, sp_w,
           bn_gamma, bn_beta, bn_mean, bn_var,
           fc1, mlp_scale, mlp_bias, fc2, w2):
    global LAST_EXEC_NS
    x = np.asarray(x, np.float32)
    r, G = _host_routing_G(x, cw0, cw1, cw2, sp_w, bn_gamma, bn_beta,
                           bn_mean, bn_var, fc1, mlp_scale, mlp_bias, fc2,
                           soa1_scale)
    consts = _consts_f16(w1, w2)
    in_maps = []
    for b in range(B):
        xT = np.ascontiguousarray(x[b].reshape(S, DIM).T).astype(np.float16)
        m = dict(consts)
        m["xt"] = xT
        gr = G[b].real.reshape(128, NBLK, FDB)
        gi = G[b].imag.reshape(128, NBLK, FDB)
        ggb = np.empty((128, NBLK, 2 * FDB), np.float16)
        ggb[:, :, 0:FDB] = gr
        ggb[:, :, FDB:2 * FDB] = gi
        m["gg"] = np.ascontiguousarray(ggb.reshape(128, 2 * 192 * 65))
        in_maps.append(m)

    if "nc" not in _CACHE:
        _CACHE["nc"] = _build()
    nc = _CACHE["nc"]
    res = run_bass_kernel_spmd(nc, in_maps, list(range(B)))
    if os.environ.get("KERNEL_TRACE_EXEC") and "tl" not in _CACHE:
        from concourse.timeline_sim import TimelineSim
        _CACHE["tl"] = TimelineSim(nc, trace=False).simulate()
    if _CACHE.get("tl"):
        LAST_EXEC_NS = int(_CACHE["tl"])

    outs = np.empty((B, H, W, DIM), np.float32)
    for b in range(B):
        o = res.results[b]["out"].astype(np.float32)      # [c, hw]
        outs[b] = o.T.reshape(H, W, DIM)

    bias = float(np.asarray(soa1_bias).reshape(-1)[0])
    if bias != 0.0:
        comb00 = G[:, 0, :, 0]                            # [B, m]
        corr = np.real(comb00).astype(np.float64) @ np.asarray(w2, np.float64).T
        outs = outs + bias * corr[:, None, None, :].astype(np.float32)
    return outs


# revision 4
# speedup vs baseline: 1.0942x; 1.0338x over previous
"""Fully-fused single-launch TRN2 kernel for nn_Decoder_1700807049879.

Per core (1 sample), everything on device except the tiny routing MLP
(host, f64) whose softmax weights are folded into the uploaded spectral
filter G:

  S1  pw1 + relu^2    -> x_pre [w, h*192+m]           (fp16 SBUF)
  S2  rfft-W (packed) -> XwT [kwf, m*128+h]; xbar T -> Xw [h, m*128+kwf]
  S3/M/S5 interleaved per 16-channel block:
      DFT-H -> Xf_blk; Y_blk = Xf_blk*G_blk; iDFT-H -> Zh [h, m*128+fold]
  T1  xbar transpose  -> Zfold [kwf, m*128+h]
  S6  irfft-W folded  -> xspA [w, h*128+m(0:128)], xspB [w, h*128+(m-96)]
  T2  xbar transposes -> dstA [m, h*128+w], dstB
  S7  pw2             -> out [c, h*128+w] -> DRAM

rfft folding: for real rows Im(kw=0)=Im(kw=64)=0, so the W-stage packs
[Cw(0:65) | -Sw(1:64)] into one 128-wide stationary and the inverse
packs [alpha*cos ; -alpha*sin] into one 128-deep contraction.
"""

import os
import sys
import numpy as np
from contextlib import ExitStack

sys.path.insert(0, "/opt/trn_rl_repo")

from concourse import bass, bacc, mybir, tile  # noqa: E402
from concourse.bass_utils import run_bass_kernel_spmd  # noqa: E402

B, H, W, DIM = 8, 128, 128, 96
MED, NS, SCTX = 192, 3, 48
FH, FWH = 128, 65
SCALE_HW = [(16, 9), (8, 4), (24, 13)]
S = H * W
F16 = mybir.dt.float16
F32 = mybir.dt.float32

MBLK = 12              # channels per S3/M/S5 block (== transpose chunk)
NBLK = MED // MBLK     # 16
FDB = MBLK * 65        # 780
GCH = 6                # channels per H-DFT psum group (N=390 <= 512)
CCH = 12               # channels per transpose chunk (1 block)


# ---------------------------------------------------------------- host math
def _cubic(t, a=-0.75):
    t = abs(t)
    if t <= 1.0:
        return (a + 2) * t ** 3 - (a + 3) * t ** 2 + 1.0
    if t < 2.0:
        return a * t ** 3 - 5 * a * t ** 2 + 8 * a * t - 4 * a
    return 0.0


def _resize_mat(old, new):
    M = np.zeros((new, old), dtype=np.float64)
    for j in range(new):
        s = j * (old - 1) / (new - 1) if new > 1 else 0.0
        f = int(np.floor(s))
        for k in range(-1, 3):
            M[j, min(max(f + k, 0), old - 1)] += _cubic(s - (f + k))
    return M


def _dft_consts():
    k = np.arange(128)
    ang = 2 * np.pi * np.outer(k, k) / 128.0
    C = np.cos(ang) / np.sqrt(128.0)
    Sm = np.sin(ang) / np.sqrt(128.0)
    CWF = np.concatenate([C[:, 0:65], -Sm[:, 1:64]], axis=1)
    alpha = np.ones(65)
    alpha[1:64] = 2.0
    cwa = (alpha[:, None]
           * np.cos(2 * np.pi * np.outer(np.arange(65), k) / 128.0)
           / np.sqrt(128.0))
    swa = (2.0 * np.sin(2 * np.pi * np.outer(np.arange(1, 64), k) / 128.0)
           / np.sqrt(128.0))
    IWF = np.concatenate([cwa, -swa], axis=0)
    return (CWF.astype(np.float16), C.astype(np.float16),
            Sm.astype(np.float16), (-Sm).astype(np.float16),
            IWF.astype(np.float16))


def _host_routing_G(x, cw0, cw1, cw2, sp_w, bn_gamma, bn_beta, bn_mean,
                    bn_var, fc1, mlp_scale, mlp_bias, fc2, soa1_scale):
    x = np.asarray(x, np.float64)
    gctx = x.mean(axis=(1, 2))
    y = np.einsum('bhwc,sc->bhws', x, np.asarray(sp_w, np.float64))
    y = ((y - np.asarray(bn_mean, np.float64))
         / np.sqrt(np.asarray(bn_var, np.float64) + 1e-5)
         * np.asarray(bn_gamma, np.float64) + np.asarray(bn_beta, np.float64))
    sctx = np.maximum(y, 0.0).mean(axis=(1, 2))
    fused = np.concatenate([gctx, sctx], axis=1)
    hm = fused @ np.asarray(fc1, np.float64).T
    ms = float(np.asarray(mlp_scale).reshape(-1)[0])
    mb = float(np.asarray(mlp_bias).reshape(-1)[0])
    hmid = ms * np.maximum(hm, 0.0) ** 2 + mb
    logits = (hmid @ np.asarray(fc2, np.float64).T).reshape(B, NS, MED)
    e = np.exp(logits - logits.max(axis=1, keepdims=True))
    r = e / e.sum(axis=1, keepdims=True)
    filts = []
    for cw, (sh, sw) in zip((cw0, cw1, cw2), SCALE_HW):
        cw = np.asarray(cw, np.float64)
        Rh = _resize_mat(sh, FH).astype(np.float32)
        Rw = _resize_mat(sw, FWH).astype(np.float32)
        t = np.einsum('Vw,hwmc->hVmc', Rw, cw.astype(np.float32))
        t = np.einsum('Hh,hVmc->HVmc', Rh, t)
        filts.append((t[..., 0] + 1j * t[..., 1]).astype(np.complex64))
    filt = np.stack(filts)                                  # [3, kh, kw, m]
    sc = float(np.asarray(soa1_scale).reshape(-1)[0])
    G = np.einsum('skwm,bsm->bkmw', filt,
                  r.astype(np.complex64)) * sc              # [B, kh, m, kw]
    return r, G


# ---------------------------------------------------------------- device
def _build():
    nc = bacc.Bacc("TRN2", target_bir_lowering=False, debug=False,
                   num_devices=B)
    xt = nc.dram_tensor("xt", [DIM, S], F16, kind="ExternalInput").ap()
    gg = nc.dram_tensor("gg", [128, 2 * 192 * 65], F16, kind="ExternalInput").ap()
    w1x_d = nc.dram_tensor("w1x", [96, 192], F16, kind="ExternalInput").ap()
    cwf_d = nc.dram_tensor("cwf", [128, 128], F16, kind="ExternalInput").ap()
    cht_d = nc.dram_tensor("cht", [128, 128], F16, kind="ExternalInput").ap()
    sht_d = nc.dram_tensor("sht", [128, 128], F16, kind="ExternalInput").ap()
    nsht_d = nc.dram_tensor("nsht", [128, 128], F16, kind="ExternalInput").ap()
    iwf_d = nc.dram_tensor("iwf", [128, 128], F16, kind="ExternalInput").ap()
    w2a_d = nc.dram_tensor("w2a", [96, 96], F16, kind="ExternalInput").ap()
    w2b_d = nc.dram_tensor("w2b", [96, 96], F16, kind="ExternalInput").ap()
    out_d = nc.dram_tensor("out", [DIM, S], F16, kind="ExternalOutput").ap()

    with tile.TileContext(nc) as tc, ExitStack() as ctx:
        cpool = ctx.enter_context(tc.tile_pool(name="c", bufs=1))
        big = ctx.enter_context(tc.tile_pool(name="big", bufs=2))
        chp = ctx.enter_context(tc.tile_pool(name="ch", bufs=3))
        spc = ctx.enter_context(tc.tile_pool(name="sc", bufs=3))
        xpool = ctx.enter_context(tc.tile_pool(name="xp", bufs=2))
        fpool = ctx.enter_context(tc.tile_pool(name="fp", bufs=3))
        gpool = ctx.enter_context(tc.tile_pool(name="gp", bufs=4))
        tpool = ctx.enter_context(tc.tile_pool(name="tp", bufs=2))
        rpool = ctx.enter_context(tc.tile_pool(name="rp", bufs=3))
        opool = ctx.enter_context(tc.tile_pool(name="op", bufs=2))
        pp = ctx.enter_context(tc.tile_pool(name="ps", bufs=6, space="PSUM"))
        pp7 = ctx.enter_context(tc.tile_pool(name="p7", bufs=2, space="PSUM"))

        w1x = cpool.tile([96, 192], F16)
        cwf = cpool.tile([128, 128], F16, tag="cwf")
        cht = cpool.tile([128, 128], F16, tag="cht")
        sht = cpool.tile([128, 128], F16, tag="sht")
        nsht = cpool.tile([128, 128], F16, tag="nsht")
        iwf = cpool.tile([128, 128], F16, tag="iwf")
        w2a = cpool.tile([96, 96], F16, tag="w2a")
        w2b = cpool.tile([96, 96], F16, tag="w2b")
        nc.sync.dma_start(w1x[:], w1x_d[:])
        nc.sync.dma_start(cwf[:], cwf_d[:])
        nc.sync.dma_start(cht[:], cht_d[:])
        nc.sync.dma_start(sht[:], sht_d[:])
        nc.sync.dma_start(nsht[:], nsht_d[:])
        nc.sync.dma_start(iwf[:], iwf_d[:])
        nc.sync.dma_start(w2a[:], w2a_d[:])
        nc.sync.dma_start(w2b[:], w2b_d[:])

        BW = 24576          # big tile free width
        SCH = 24            # channels per S2 chunk (2 blocks)
        SCW = SCH * 128     # 3072
        TCH = 24            # channels per T1 chunk (2 blocks)
        TCW = TCH * 128     # 3072

        # ---------------- S1: pw1 + relu^2 -> x_pre [w, h*192+m]
        x_pre = big.tile([128, BW], F16, tag="big")
        XCH = 1024  # xt chunk cols (8 h)
        for ci in range(S // XCH):
            xc = xpool.tile([96, XCH], F16, tag="xt")
            nc.sync.dma_start(xc[:], xt[:, ci * XCH:(ci + 1) * XCH])
            for hh in range(0, 8, 2):
                h = ci * 8 + hh
                ps = pp.tile([128, 512], F32, tag="ps")
                for j in range(2):
                    nc.tensor.matmul(
                        ps[:, j * 192:(j + 1) * 192],
                        xc[:, (hh + j) * 128:(hh + j + 1) * 128],
                        w1x[:], start=True, stop=True)
                # relu^2: alternate (ACT relu, DVE sq) / (DVE max0, ACT sq)
                rt = rpool.tile([128, 384], F16, tag="rt")
                if (h // 2) % 2 == 0:
                    nc.scalar.activation(rt[:], ps[:, 0:384],
                                         mybir.ActivationFunctionType.Relu)
                    nc.vector.tensor_mul(x_pre[:, h * 192:(h + 2) * 192],
                                         rt[:], rt[:])
                else:
                    nc.vector.tensor_scalar_max(rt[:], ps[:, 0:384], 0.0)
                    nc.gpsimd.tensor_mul(x_pre[:, h * 192:(h + 2) * 192],
                                         rt[:], rt[:])

        # ---------------- S2: rfft-W packed, chunked (48 ch) -> Xw chunks
        xp3 = x_pre[:].rearrange("p (h m) -> p m h", h=128, m=192)
        xw_chunks = {}

        def s2_chunk(c):
            xwt = chp.tile([128, SCW], F16, tag="xwt")
            for jj in range(SCH // 4):            # 6 matmuls of N=512
                j = c * (SCH // 4) + jj
                ps = pp.tile([128, 512], F32, tag="ps")
                nc.tensor.matmul(ps[:], cwf[:],
                                 xp3[:, j * 4:(j + 1) * 4, :],
                                 start=True, stop=True)
                if j % 2 == 0:
                    nc.scalar.copy(xwt[:, jj * 512:(jj + 1) * 512], ps[:])
                else:
                    nc.vector.tensor_copy(xwt[:, jj * 512:(jj + 1) * 512],
                                          ps[:])
            xw = chp.tile([128, SCW], F16, tag="xw")
            nc.sync.dma_start_transpose(
                xw[:].rearrange("p (j k) -> p j k", j=SCH, k=128), xwt[:])
            xw_chunks[c] = xw

        # ---------------- S3/M/S5 software-pipelined over 12-channel blocks
        Zf = big.tile([128, BW], F16, tag="big")
        zf3d = Zf[:].rearrange("p (j k) -> p j k", j=192, k=128)
        yts = {}
        zhs = {}

        def s3_and_m(blk):
            m0 = blk * MBLK
            xw3 = xw_chunks[blk // 2][:].rearrange(
                "p (m c) -> p m c", m=SCH, c=128)
            lm0 = (blk % 2) * MBLK
            gt = gpool.tile([128, 2 * FDB], F16, tag="g")
            nc.gpsimd.dma_start(gt[:], gg[:, blk * 2 * FDB:(blk + 1) * 2 * FDB])
            xf = fpool.tile([128, 2 * FDB], F16, tag="xf")
            for q in range(MBLK // GCH):
                g0 = lm0 + q * GCH
                NW = GCH * 65                     # 390
                re_in = xw3[:, g0:g0 + GCH, 0:65]
                im_in = xw3[:, g0:g0 + GCH, 65:128]
                psr = pp.tile([128, 512], F32, tag="ps")
                psi = pp.tile([128, 512], F32, tag="ps")
                psr3 = psr[:, 0:NW].rearrange("p (m c) -> p m c", m=GCH, c=65)
                psi3 = psi[:, 0:NW].rearrange("p (m c) -> p m c", m=GCH, c=65)
                nc.tensor.matmul(psr[:, 0:NW], cht[:], re_in,
                                 start=True, stop=False)
                nc.tensor.matmul(psr3[:, :, 1:64], sht[:], im_in,
                                 start=False, stop=True)
                nc.tensor.matmul(psi[:, 0:NW], nsht[:], re_in,
                                 start=True, stop=False)
                nc.tensor.matmul(psi3[:, :, 1:64], cht[:], im_in,
                                 start=False, stop=True)
                o = q * NW
                nc.scalar.copy(xf[:, o:o + NW], psr[:, 0:NW])
                nc.vector.tensor_copy(xf[:, FDB + o:FDB + o + NW],
                                      psi[:, 0:NW])
            # M: Y = Xf * G (1 mul POOL, 3 muls + combines DVE)
            xfre, xfim = xf[:, 0:FDB], xf[:, FDB:2 * FDB]
            gre_t, gim_t = gt[:, 0:FDB], gt[:, FDB:2 * FDB]
            yt = fpool.tile([128, 2 * FDB], F16, tag="y")
            t1 = tpool.tile([128, FDB], F16, tag="mt")
            nc.vector.tensor_mul(yt[:, 0:FDB], xfre[:], gre_t[:])
            nc.vector.tensor_mul(t1[:], xfim[:], gim_t[:])
            nc.gpsimd.tensor_mul(yt[:, FDB:2 * FDB], xfim[:], gre_t[:])
            nc.vector.tensor_sub(yt[:, 0:FDB], yt[:, 0:FDB], t1[:])
            t2 = tpool.tile([128, FDB], F16, tag="mt")
            nc.vector.tensor_mul(t2[:], xfre[:], gim_t[:])
            nc.vector.tensor_add(yt[:, FDB:2 * FDB], yt[:, FDB:2 * FDB], t2[:])
            yts[blk] = yt

        def s5(blk):
            if blk % 2 == 0:
                zhc = chp.tile([128, TCW], F16, tag="zh")
                zhs[blk // 2] = zhc
            zh = zhs[blk // 2]
            lm0 = (blk % 2) * MBLK
            yt = yts.pop(blk)
            yre3 = yt[:, 0:FDB].rearrange("p (m c) -> p m c", m=MBLK, c=65)
            yim3 = yt[:, FDB:2 * FDB].rearrange("p (m c) -> p m c",
                                                m=MBLK, c=65)
            for q in range(MBLK // GCH):
                q0 = q * GCH
                NW = GCH * 65                     # 390
                NI = GCH * 63                     # 378
                re_in = yre3[:, q0:q0 + GCH, :]
                im_in = yim3[:, q0:q0 + GCH, :]
                psr = pp.tile([128, 512], F32, tag="ps")
                psi = pp.tile([128, 512], F32, tag="ps")
                nc.tensor.matmul(psr[:, 0:NW], cht[:], re_in,
                                 start=True, stop=False)
                nc.tensor.matmul(psr[:, 0:NW], nsht[:], im_in,
                                 start=False, stop=True)
                nc.tensor.matmul(psi[:, 0:NI], sht[:],
                                 yre3[:, q0:q0 + GCH, 1:64],
                                 start=True, stop=False)
                nc.tensor.matmul(psi[:, 0:NI], cht[:],
                                 yim3[:, q0:q0 + GCH, 1:64],
                                 start=False, stop=True)
                zh3 = zh[:, (lm0 + q0) * 128:(lm0 + q0 + GCH) * 128].rearrange(
                    "p (m c) -> p m c", m=GCH, c=128)
                psr3 = psr[:, 0:NW].rearrange("p (m c) -> p m c", m=GCH, c=65)
                psi3 = psi[:, 0:NI].rearrange("p (m c) -> p m c", m=GCH, c=63)
                nc.scalar.copy(zh3[:, :, 0:65], psr3[:])
                nc.scalar.copy(zh3[:, :, 65:128], psi3[:])
            if blk % 2 == 1:        # 24 channels done -> T1 chunk
                c = blk // 2
                nc.sync.dma_start_transpose(
                    zf3d[:, c * TCH:(c + 1) * TCH, :], zhs.pop(c)[:])

        s2_chunk(0)
        s2_chunk(1)
        for blk in range(NBLK):
            if blk % 2 == 0 and blk // 2 + 2 < MED // SCH:
                s2_chunk(blk // 2 + 2)
            s3_and_m(blk)
            if blk >= 2:
                s5(blk - 2)
        s5(NBLK - 2)
        s5(NBLK - 1)

        # ---------------- S6 (stationary-data irfft-W, no transpose) + S7
        # per 4-h group: lhsT = Zf[:, m*128+h] slices -> psum [m-chunk, 4*128]
        def s6_s7(hb):
            h4 = hb * 4
            dd = []
            for base in (0, 12288):              # m 0..95 | m 96..191
                ps = pp.tile([128, 512], F32, tag="ps")
                for k in range(4):
                    h = h4 + k
                    nc.tensor.matmul(
                        ps[0:96, k * 128:(k + 1) * 128],
                        Zf[:, base + h:base + h + 95 * 128 + 1:128],
                        iwf[:], start=True, stop=True)
                d = spc.tile([96, 512], F16, tag="dA" if base == 0 else "dB")
                if hb % 2 == 0:
                    nc.scalar.copy(d[:], ps[0:96, :])
                else:
                    nc.vector.tensor_copy(d[:], ps[0:96, :])
                dd.append(d)
            ps7 = pp7.tile([128, 512], F32, tag="p7")
            nc.tensor.matmul(ps7[0:96, :], w2a[:], dd[0][:],
                             start=True, stop=False)
            nc.tensor.matmul(ps7[0:96, :], w2b[:], dd[1][:],
                             start=False, stop=True)
            if hb % 4 == 0:
                obt = opool.tile([96, 2048], F16, tag="ob")
                obs[0] = obt
            ob = obs[0]
            sl = slice((hb % 4) * 512, (hb % 4 + 1) * 512)
            if hb % 2 == 0:
                nc.vector.tensor_copy(ob[:, sl], ps7[0:96, :])
            else:
                nc.scalar.copy(ob[:, sl], ps7[0:96, :])
            if hb % 4 == 3:
                nc.scalar.dma_start(
                    out_d[:, (h4 - 12) * 128:(h4 + 4) * 128], ob[:])

        obs = [None]
        for hb in range(32):
            s6_s7(hb)
    nc.finalize()
    return nc


_CACHE = {}
LAST_EXEC_NS = 0


def _consts_f16(w1, w2):
    CWF, CHT, SHT, NSHT, IWF = _dft_consts()
    return dict(
        w1x=np.ascontiguousarray(np.asarray(w1, np.float32).T).astype(np.float16),
        cwf=CWF, cht=CHT, sht=SHT, nsht=NSHT, iwf=IWF,
        w2a=np.ascontiguousarray(np.asarray(w2, np.float32).T[0:96]).astype(np.float16),
        w2b=np.ascontiguousarray(np.asarray(w2, np.float32).T[96:192]).astype(np.float16),
    )


def kernel(x, w1, soa1_scale, soa1_bias, cw0, cw1, cw2, sp_w,
           bn_gamma, bn_beta, bn_mean, bn_var,
           fc1, mlp_scale, mlp_bias, fc2, w2):
    global LAST_EXEC_NS
    x = np.asarray(x, np.float32)
    r, G = _host_routing_G(x, cw0, cw1, cw2, sp_w, bn_gamma, bn_beta,
                           bn_mean, bn_var, fc1, mlp_scale, mlp_bias, fc2,
                           soa1_scale)
    consts = _consts_f16(w1, w2)
    in_maps = []
    for b in range(B):
        xT = np.ascontiguousarray(x[b].reshape(S, DIM).T).astype(np.float16)
        m = dict(consts)
        m["xt"] = xT
        gr = G[b].real.reshape(128, NBLK, FDB)
        gi = G[b].imag.reshape(128, NBLK, FDB)
        ggb = np.empty((128, NBLK, 2 * FDB), np.float16)
        ggb[:, :, 0:FDB] = gr
        ggb[:, :, FDB:2 * FDB] = gi
        m["gg"] = np.ascontiguousarray(ggb.reshape(128, 2 * 192 * 65))
        in_maps.append(m)

    if "nc" not in _CACHE:
        _CACHE["nc"] = _build()
    nc = _CACHE["nc"]
    res = run_bass_kernel_spmd(nc, in_maps, list(range(B)))
    if os.environ.get("KERNEL_TRACE_EXEC") and "tl" not in _CACHE:
        from concourse.timeline_sim import TimelineSim
        _CACHE["tl"] = TimelineSim(nc, trace=False).simulate()
    if _CACHE.get("tl"):
        LAST_EXEC_NS = int(_CACHE["tl"])

    outs = np.empty((B, H, W, DIM), np.float32)
    for b in range(B):
        o = res.results[b]["out"].astype(np.float32)      # [c, hw]
        outs[b] = o.T.reshape(H, W, DIM)

    bias = float(np.asarray(soa1_bias).reshape(-1)[0])
    if bias != 0.0:
        comb00 = G[:, 0, :, 0]                            # [B, m]
        corr = np.real(comb00).astype(np.float64) @ np.asarray(w2, np.float64).T
        outs = outs + bias * corr[:, None, None, :].astype(np.float32)
    return outs


# revision 5
# speedup vs baseline: 1.2019x; 1.0984x over previous
"""Fully-fused single-launch TRN2 kernel for nn_Decoder_1700807049879.

Per core (1 sample), everything on device except the tiny routing MLP
(host, f64) whose softmax weights are folded into the uploaded spectral
filter G:

  S1  pw1 + relu^2    -> x_pre [w, h*192+m]           (fp16 SBUF)
  S2  rfft-W (packed) -> XwT [kwf, m*128+h]; xbar T -> Xw [h, m*128+kwf]
  S3/M/S5 interleaved per 16-channel block:
      DFT-H -> Xf_blk; Y_blk = Xf_blk*G_blk; iDFT-H -> Zh [h, m*128+fold]
  T1  xbar transpose  -> Zfold [kwf, m*128+h]
  S6  irfft-W folded  -> xspA [w, h*128+m(0:128)], xspB [w, h*128+(m-96)]
  T2  xbar transposes -> dstA [m, h*128+w], dstB
  S7  pw2             -> out [c, h*128+w] -> DRAM

rfft folding: for real rows Im(kw=0)=Im(kw=64)=0, so the W-stage packs
[Cw(0:65) | -Sw(1:64)] into one 128-wide stationary and the inverse
packs [alpha*cos ; -alpha*sin] into one 128-deep contraction.
"""

import os
import sys
import numpy as np
from contextlib import ExitStack

sys.path.insert(0, "/opt/trn_rl_repo")

from concourse import bass, bacc, mybir, tile  # noqa: E402
from concourse.bass_utils import run_bass_kernel_spmd  # noqa: E402

B, H, W, DIM = 8, 128, 128, 96
MED, NS, SCTX = 192, 3, 48
FH, FWH = 128, 65
SCALE_HW = [(16, 9), (8, 4), (24, 13)]
S = H * W
F16 = mybir.dt.float16
F32 = mybir.dt.float32

MBLK = 12              # channels per S3/M/S5 block (== transpose chunk)
NBLK = MED // MBLK     # 16
FDB = MBLK * 65        # 780
GCH = 6                # channels per H-DFT psum group (N=390 <= 512)
CCH = 12               # channels per transpose chunk (1 block)


# ---------------------------------------------------------------- host math
def _cubic(t, a=-0.75):
    t = abs(t)
    if t <= 1.0:
        return (a + 2) * t ** 3 - (a + 3) * t ** 2 + 1.0
    if t < 2.0:
        return a * t ** 3 - 5 * a * t ** 2 + 8 * a * t - 4 * a
    return 0.0


def _resize_mat(old, new):
    M = np.zeros((new, old), dtype=np.float64)
    for j in range(new):
        s = j * (old - 1) / (new - 1) if new > 1 else 0.0
        f = int(np.floor(s))
        for k in range(-1, 3):
            M[j, min(max(f + k, 0), old - 1)] += _cubic(s - (f + k))
    return M


def _dft_consts():
    k = np.arange(128)
    ang = 2 * np.pi * np.outer(k, k) / 128.0
    C = np.cos(ang) / np.sqrt(128.0)
    Sm = np.sin(ang) / np.sqrt(128.0)
    CWF = np.concatenate([C[:, 0:65], -Sm[:, 1:64]], axis=1)
    alpha = np.ones(65)
    alpha[1:64] = 2.0
    cwa = (alpha[:, None]
           * np.cos(2 * np.pi * np.outer(np.arange(65), k) / 128.0)
           / np.sqrt(128.0))
    swa = (2.0 * np.sin(2 * np.pi * np.outer(np.arange(1, 64), k) / 128.0)
           / np.sqrt(128.0))
    IWF = np.concatenate([cwa, -swa], axis=0)
    return (CWF.astype(np.float16), C.astype(np.float16),
            Sm.astype(np.float16), (-Sm).astype(np.float16),
            IWF.astype(np.float16))


def _host_routing_G(x, cw0, cw1, cw2, sp_w, bn_gamma, bn_beta, bn_mean,
                    bn_var, fc1, mlp_scale, mlp_bias, fc2, soa1_scale):
    x = np.asarray(x, np.float64)
    gctx = x.mean(axis=(1, 2))
    y = np.einsum('bhwc,sc->bhws', x, np.asarray(sp_w, np.float64))
    y = ((y - np.asarray(bn_mean, np.float64))
         / np.sqrt(np.asarray(bn_var, np.float64) + 1e-5)
         * np.asarray(bn_gamma, np.float64) + np.asarray(bn_beta, np.float64))
    sctx = np.maximum(y, 0.0).mean(axis=(1, 2))
    fused = np.concatenate([gctx, sctx], axis=1)
    hm = fused @ np.asarray(fc1, np.float64).T
    ms = float(np.asarray(mlp_scale).reshape(-1)[0])
    mb = float(np.asarray(mlp_bias).reshape(-1)[0])
    hmid = ms * np.maximum(hm, 0.0) ** 2 + mb
    logits = (hmid @ np.asarray(fc2, np.float64).T).reshape(B, NS, MED)
    e = np.exp(logits - logits.max(axis=1, keepdims=True))
    r = e / e.sum(axis=1, keepdims=True)
    filts = []
    for cw, (sh, sw) in zip((cw0, cw1, cw2), SCALE_HW):
        cw = np.asarray(cw, np.float64)
        Rh = _resize_mat(sh, FH).astype(np.float32)
        Rw = _resize_mat(sw, FWH).astype(np.float32)
        t = np.einsum('Vw,hwmc->hVmc', Rw, cw.astype(np.float32))
        t = np.einsum('Hh,hVmc->HVmc', Rh, t)
        filts.append((t[..., 0] + 1j * t[..., 1]).astype(np.complex64))
    filt = np.stack(filts)                                  # [3, kh, kw, m]
    sc = float(np.asarray(soa1_scale).reshape(-1)[0])
    G = np.einsum('skwm,bsm->bkmw', filt,
                  r.astype(np.complex64)) * sc              # [B, kh, m, kw]
    return r, G


# ---------------------------------------------------------------- device
def _build():
    nc = bacc.Bacc("TRN2", target_bir_lowering=False, debug=False,
                   num_devices=B)
    xt = nc.dram_tensor("xt", [DIM, S], F16, kind="ExternalInput").ap()
    gg = nc.dram_tensor("gg", [128, 2 * 192 * 65], F16, kind="ExternalInput").ap()
    w1x_d = nc.dram_tensor("w1x", [96, 192], F16, kind="ExternalInput").ap()
    cwf_d = nc.dram_tensor("cwf", [128, 128], F16, kind="ExternalInput").ap()
    cht_d = nc.dram_tensor("cht", [128, 128], F16, kind="ExternalInput").ap()
    sht_d = nc.dram_tensor("sht", [128, 128], F16, kind="ExternalInput").ap()
    nsht_d = nc.dram_tensor("nsht", [128, 128], F16, kind="ExternalInput").ap()
    iwf_d = nc.dram_tensor("iwf", [128, 128], F16, kind="ExternalInput").ap()
    w2a_d = nc.dram_tensor("w2a", [96, 96], F16, kind="ExternalInput").ap()
    w2b_d = nc.dram_tensor("w2b", [96, 96], F16, kind="ExternalInput").ap()
    out_d = nc.dram_tensor("out", [DIM, S], F16, kind="ExternalOutput").ap()

    with tile.TileContext(nc) as tc, ExitStack() as ctx:
        cpool = ctx.enter_context(tc.tile_pool(name="c", bufs=1))
        big = ctx.enter_context(tc.tile_pool(name="big", bufs=2))
        chp = ctx.enter_context(tc.tile_pool(name="ch", bufs=3))
        spc = ctx.enter_context(tc.tile_pool(name="sc", bufs=3))
        xpool = ctx.enter_context(tc.tile_pool(name="xp", bufs=2))
        fpool = ctx.enter_context(tc.tile_pool(name="fp", bufs=3))
        gpool = ctx.enter_context(tc.tile_pool(name="gp", bufs=4))
        tpool = ctx.enter_context(tc.tile_pool(name="tp", bufs=2))
        rpool = ctx.enter_context(tc.tile_pool(name="rp", bufs=3))
        opool = ctx.enter_context(tc.tile_pool(name="op", bufs=2))
        pp = ctx.enter_context(tc.tile_pool(name="ps", bufs=6, space="PSUM"))
        pp7 = ctx.enter_context(tc.tile_pool(name="p7", bufs=2, space="PSUM"))

        w1x = cpool.tile([96, 192], F16)
        cwf = cpool.tile([128, 128], F16, tag="cwf")
        cht = cpool.tile([128, 128], F16, tag="cht")
        sht = cpool.tile([128, 128], F16, tag="sht")
        nsht = cpool.tile([128, 128], F16, tag="nsht")
        iwf = cpool.tile([128, 128], F16, tag="iwf")
        w2a = cpool.tile([96, 96], F16, tag="w2a")
        w2b = cpool.tile([96, 96], F16, tag="w2b")
        nc.sync.dma_start(w1x[:], w1x_d[:])
        nc.sync.dma_start(cwf[:], cwf_d[:])
        nc.sync.dma_start(cht[:], cht_d[:])
        nc.sync.dma_start(sht[:], sht_d[:])
        nc.sync.dma_start(nsht[:], nsht_d[:])
        nc.sync.dma_start(iwf[:], iwf_d[:])
        nc.sync.dma_start(w2a[:], w2a_d[:])
        nc.sync.dma_start(w2b[:], w2b_d[:])

        BW = 24576          # big tile free width
        SCH = 24            # channels per S2 chunk (2 blocks)
        SCW = SCH * 128     # 3072
        TCH = 24            # channels per T1 chunk (2 blocks)
        TCW = TCH * 128     # 3072

        # ---------------- S1: pw1 + relu^2 -> x_pre [w, h*192+m]
        x_pre = big.tile([128, BW], F16, tag="big")
        XCH = 1024  # xt chunk cols (8 h)
        for ci in range(S // XCH):
            xc = xpool.tile([96, XCH], F16, tag="xt")
            nc.sync.dma_start(xc[:], xt[:, ci * XCH:(ci + 1) * XCH])
            for hh in range(0, 8, 2):
                h = ci * 8 + hh
                ps = pp.tile([128, 512], F32, tag="ps")
                for j in range(2):
                    nc.tensor.matmul(
                        ps[:, j * 192:(j + 1) * 192],
                        xc[:, (hh + j) * 128:(hh + j + 1) * 128],
                        w1x[:], start=True, stop=True)
                # relu^2: alternate (ACT relu, DVE sq) / (DVE max0, ACT sq)
                rt = rpool.tile([128, 384], F16, tag="rt")
                if (h // 2) % 2 == 0:
                    nc.scalar.activation(rt[:], ps[:, 0:384],
                                         mybir.ActivationFunctionType.Relu)
                    nc.vector.tensor_mul(x_pre[:, h * 192:(h + 2) * 192],
                                         rt[:], rt[:])
                else:
                    nc.vector.tensor_scalar_max(rt[:], ps[:, 0:384], 0.0)
                    nc.gpsimd.tensor_mul(x_pre[:, h * 192:(h + 2) * 192],
                                         rt[:], rt[:])

        # ---------------- S2: rfft-W packed, chunked (48 ch) -> Xw chunks
        xp3 = x_pre[:].rearrange("p (h m) -> p m h", h=128, m=192)
        xw_chunks = {}

        def s2_chunk(c):
            xwt = chp.tile([128, SCW], F16, tag="xwt")
            for jj in range(SCH // 4):            # 6 matmuls of N=512
                j = c * (SCH // 4) + jj
                ps = pp.tile([128, 512], F32, tag="ps")
                nc.tensor.matmul(ps[:], cwf[:],
                                 xp3[:, j * 4:(j + 1) * 4, :],
                                 start=True, stop=True)
                if j % 2 == 0:
                    nc.scalar.copy(xwt[:, jj * 512:(jj + 1) * 512], ps[:])
                else:
                    nc.vector.tensor_copy(xwt[:, jj * 512:(jj + 1) * 512],
                                          ps[:])
            xw = chp.tile([128, SCW], F16, tag="xw")
            nc.sync.dma_start_transpose(
                xw[:].rearrange("p (j k) -> p j k", j=SCH, k=128), xwt[:])
            xw_chunks[c] = xw

        # ---------------- S3/M/S5 software-pipelined over 12-channel blocks
        Zf = big.tile([128, BW], F16, tag="big")
        zf3d = Zf[:].rearrange("p (j k) -> p j k", j=192, k=128)
        yts = {}
        zhs = {}

        def s3_and_m(blk):
            m0 = blk * MBLK
            xw3 = xw_chunks[blk // 2][:].rearrange(
                "p (m c) -> p m c", m=SCH, c=128)
            lm0 = (blk % 2) * MBLK
            gt = gpool.tile([128, 2 * FDB], F16, tag="g")
            nc.gpsimd.dma_start(gt[:], gg[:, blk * 2 * FDB:(blk + 1) * 2 * FDB])
            xf = fpool.tile([128, 2 * FDB], F16, tag="xf")
            for q in range(MBLK // GCH):
                g0 = lm0 + q * GCH
                NW = GCH * 65                     # 390
                re_in = xw3[:, g0:g0 + GCH, 0:65]
                im_in = xw3[:, g0:g0 + GCH, 65:128]
                psr = pp.tile([128, 512], F32, tag="ps")
                psi = pp.tile([128, 512], F32, tag="ps")
                psr3 = psr[:, 0:NW].rearrange("p (m c) -> p m c", m=GCH, c=65)
                psi3 = psi[:, 0:NW].rearrange("p (m c) -> p m c", m=GCH, c=65)
                nc.tensor.matmul(psr[:, 0:NW], cht[:], re_in,
                                 start=True, stop=False)
                nc.tensor.matmul(psr3[:, :, 1:64], sht[:], im_in,
                                 start=False, stop=True)
                nc.tensor.matmul(psi[:, 0:NW], nsht[:], re_in,
                                 start=True, stop=False)
                nc.tensor.matmul(psi3[:, :, 1:64], cht[:], im_in,
                                 start=False, stop=True)
                o = q * NW
                nc.scalar.copy(xf[:, o:o + NW], psr[:, 0:NW])
                nc.scalar.copy(xf[:, FDB + o:FDB + o + NW], psi[:, 0:NW])
            # M: Y = Xf * G (1 mul POOL, 3 muls + combines DVE)
            xfre, xfim = xf[:, 0:FDB], xf[:, FDB:2 * FDB]
            gre_t, gim_t = gt[:, 0:FDB], gt[:, FDB:2 * FDB]
            yt = fpool.tile([128, 2 * FDB], F16, tag="y")
            t1 = tpool.tile([128, FDB], F16, tag="mt")
            nc.vector.tensor_mul(yt[:, 0:FDB], xfre[:], gre_t[:])
            nc.vector.tensor_mul(t1[:], xfim[:], gim_t[:])
            nc.gpsimd.tensor_mul(yt[:, FDB:2 * FDB], xfim[:], gre_t[:])
            nc.vector.tensor_sub(yt[:, 0:FDB], yt[:, 0:FDB], t1[:])
            t2 = tpool.tile([128, FDB], F16, tag="mt")
            nc.vector.tensor_mul(t2[:], xfre[:], gim_t[:])
            nc.vector.tensor_add(yt[:, FDB:2 * FDB], yt[:, FDB:2 * FDB], t2[:])
            yts[blk] = yt

        def s5(blk):
            if blk % 2 == 0:
                zhc = chp.tile([128, TCW], F16, tag="zh")
                zhs[blk // 2] = zhc
            zh = zhs[blk // 2]
            lm0 = (blk % 2) * MBLK
            yt = yts.pop(blk)
            yre3 = yt[:, 0:FDB].rearrange("p (m c) -> p m c", m=MBLK, c=65)
            yim3 = yt[:, FDB:2 * FDB].rearrange("p (m c) -> p m c",
                                                m=MBLK, c=65)
            for q in range(MBLK // GCH):
                q0 = q * GCH
                NW = GCH * 65                     # 390
                NI = GCH * 63                     # 378
                re_in = yre3[:, q0:q0 + GCH, :]
                im_in = yim3[:, q0:q0 + GCH, :]
                psr = pp.tile([128, 512], F32, tag="ps")
                psi = pp.tile([128, 512], F32, tag="ps")
                nc.tensor.matmul(psr[:, 0:NW], cht[:], re_in,
                                 start=True, stop=False)
                nc.tensor.matmul(psr[:, 0:NW], nsht[:], im_in,
                                 start=False, stop=True)
                nc.tensor.matmul(psi[:, 0:NI], sht[:],
                                 yre3[:, q0:q0 + GCH, 1:64],
                                 start=True, stop=False)
                nc.tensor.matmul(psi[:, 0:NI], cht[:],
                                 yim3[:, q0:q0 + GCH, 1:64],
                                 start=False, stop=True)
                zh3 = zh[:, (lm0 + q0) * 128:(lm0 + q0 + GCH) * 128].rearrange(
                    "p (m c) -> p m c", m=GCH, c=128)
                psr3 = psr[:, 0:NW].rearrange("p (m c) -> p m c", m=GCH, c=65)
                psi3 = psi[:, 0:NI].rearrange("p (m c) -> p m c", m=GCH, c=63)
                nc.scalar.copy(zh3[:, :, 0:65], psr3[:])
                nc.scalar.copy(zh3[:, :, 65:128], psi3[:])
            if blk % 2 == 1:        # 24 channels done -> T1 chunk
                c = blk // 2
                nc.sync.dma_start_transpose(
                    zf3d[:, c * TCH:(c + 1) * TCH, :], zhs.pop(c)[:])

        s2_chunk(0)
        s2_chunk(1)
        for blk in range(NBLK):
            if blk % 2 == 0 and blk // 2 + 2 < MED // SCH:
                s2_chunk(blk // 2 + 2)
            s3_and_m(blk)
            if blk >= 2:
                s5(blk - 2)
        s5(NBLK - 2)
        s5(NBLK - 1)

        # ---------------- S6 (stationary-data irfft-W, no transpose) + S7
        # per 4-h group: lhsT = Zf[:, m*128+h] slices -> psum [m-chunk, 4*128]
        def s6_s7(hb):
            h4 = hb * 4
            dd = []
            for base in (0, 12288):              # m 0..95 | m 96..191
                ps = pp.tile([128, 512], F32, tag="ps")
                for k in range(4):
                    h = h4 + k
                    nc.tensor.matmul(
                        ps[0:96, k * 128:(k + 1) * 128],
                        Zf[:, base + h:base + h + 95 * 128 + 1:128],
                        iwf[:], start=True, stop=True)
                d = spc.tile([96, 512], F16, tag="dA" if base == 0 else "dB")
                if hb % 2 == 0:
                    nc.scalar.copy(d[:], ps[0:96, :])
                else:
                    nc.vector.tensor_copy(d[:], ps[0:96, :])
                dd.append(d)
            ps7 = pp7.tile([128, 512], F32, tag="p7")
            nc.tensor.matmul(ps7[0:96, :], w2a[:], dd[0][:],
                             start=True, stop=False)
            nc.tensor.matmul(ps7[0:96, :], w2b[:], dd[1][:],
                             start=False, stop=True)
            if hb % 4 == 0:
                obt = opool.tile([96, 2048], F16, tag="ob")
                obs[0] = obt
            ob = obs[0]
            sl = slice((hb % 4) * 512, (hb % 4 + 1) * 512)
            if hb % 2 == 0:
                nc.vector.tensor_copy(ob[:, sl], ps7[0:96, :])
            else:
                nc.scalar.copy(ob[:, sl], ps7[0:96, :])
            if hb % 4 == 3:
                nc.scalar.dma_start(
                    out_d[:, (h4 - 12) * 128:(h4 + 4) * 128], ob[:])

        obs = [None]
        for hb in range(32):
            s6_s7(hb)
    nc.finalize()
    return nc


_CACHE = {}
LAST_EXEC_NS = 0


def _consts_f16(w1, w2):
    CWF, CHT, SHT, NSHT, IWF = _dft_consts()
    return dict(
        w1x=np.ascontiguousarray(np.asarray(w1, np.float32).T).astype(np.float16),
        cwf=CWF, cht=CHT, sht=SHT, nsht=NSHT, iwf=IWF,
        w2a=np.ascontiguousarray(np.asarray(w2, np.float32).T[0:96]).astype(np.float16),
        w2b=np.ascontiguousarray(np.asarray(w2, np.float32).T[96:192]).astype(np.float16),
    )


def kernel(x, w1, soa1_scale, soa1_bias, cw0, cw1, cw2, sp_w,
           bn_gamma, bn_beta, bn_mean, bn_var,
           fc1, mlp_scale, mlp_bias, fc2, w2):
    global LAST_EXEC_NS
    x = np.asarray(x, np.float32)
    r, G = _host_routing_G(x, cw0, cw1, cw2, sp_w, bn_gamma, bn_beta,
                           bn_mean, bn_var, fc1, mlp_scale, mlp_bias, fc2,
                           soa1_scale)
    consts = _consts_f16(w1, w2)
    in_maps = []
    for b in range(B):
        xT = np.ascontiguousarray(x[b].reshape(S, DIM).T).astype(np.float16)
        m = dict(consts)
        m["xt"] = xT
        gr = G[b].real.reshape(128, NBLK, FDB)
        gi = G[b].imag.reshape(128, NBLK, FDB)
        ggb = np.empty((128, NBLK, 2 * FDB), np.float16)
        ggb[:, :, 0:FDB] = gr
        ggb[:, :, FDB:2 * FDB] = gi
        m["gg"] = np.ascontiguousarray(ggb.reshape(128, 2 * 192 * 65))
        in_maps.append(m)

    if "nc" not in _CACHE:
        _CACHE["nc"] = _build()
    nc = _CACHE["nc"]
    res = run_bass_kernel_spmd(nc, in_maps, list(range(B)))
    if os.environ.get("KERNEL_TRACE_EXEC") and "tl" not in _CACHE:
        from concourse.timeline_sim import TimelineSim
        _CACHE["tl"] = TimelineSim(nc, trace=False).simulate()
    if _CACHE.get("tl"):
        LAST_EXEC_NS = int(_CACHE["tl"])

    outs = np.empty((B, H, W, DIM), np.float32)
    for b in range(B):
        o = res.results[b]["out"].astype(np.float32)      # [c, hw]
        outs[b] = o.T.reshape(H, W, DIM)

    bias = float(np.asarray(soa1_bias).reshape(-1)[0])
    if bias != 0.0:
        comb00 = G[:, 0, :, 0]                            # [B, m]
        corr = np.real(comb00).astype(np.float64) @ np.asarray(w2, np.float64).T
        outs = outs + bias * corr[:, None, None, :].astype(np.float32)
    return outs


# revision 6
# speedup vs baseline: 1.2193x; 1.0145x over previous
"""Fully-fused single-launch TRN2 kernel for nn_Decoder_1700807049879.

Per core (1 sample), everything on device except the tiny routing MLP
(host, f64) whose softmax weights are folded into the uploaded spectral
filter G:

  S1  pw1 + relu^2    -> x_pre [w, h*192+m]           (fp16 SBUF)
  S2  rfft-W (packed) -> XwT [kwf, m*128+h]; xbar T -> Xw [h, m*128+kwf]
  S3/M/S5 interleaved per 16-channel block:
      DFT-H -> Xf_blk; Y_blk = Xf_blk*G_blk; iDFT-H -> Zh [h, m*128+fold]
  T1  xbar transpose  -> Zfold [kwf, m*128+h]
  S6  irfft-W folded  -> xspA [w, h*128+m(0:128)], xspB [w, h*128+(m-96)]
  T2  xbar transposes -> dstA [m, h*128+w], dstB
  S7  pw2             -> out [c, h*128+w] -> DRAM

rfft folding: for real rows Im(kw=0)=Im(kw=64)=0, so the W-stage packs
[Cw(0:65) | -Sw(1:64)] into one 128-wide stationary and the inverse
packs [alpha*cos ; -alpha*sin] into one 128-deep contraction.
"""

import os
import sys
import numpy as np
from contextlib import ExitStack

sys.path.insert(0, "/opt/trn_rl_repo")

from concourse import bass, bacc, mybir, tile  # noqa: E402
from concourse.bass_utils import run_bass_kernel_spmd  # noqa: E402

B, H, W, DIM = 8, 128, 128, 96
MED, NS, SCTX = 192, 3, 48
FH, FWH = 128, 65
SCALE_HW = [(16, 9), (8, 4), (24, 13)]
S = H * W
F16 = mybir.dt.float16
F32 = mybir.dt.float32

MBLK = 12              # channels per S3/M/S5 block (== transpose chunk)
NBLK = MED // MBLK     # 16
FDB = MBLK * 65        # 780
GCH = 6                # channels per H-DFT psum group (N=390 <= 512)
CCH = 12               # channels per transpose chunk (1 block)


# ---------------------------------------------------------------- host math
def _cubic(t, a=-0.75):
    t = abs(t)
    if t <= 1.0:
        return (a + 2) * t ** 3 - (a + 3) * t ** 2 + 1.0
    if t < 2.0:
        return a * t ** 3 - 5 * a * t ** 2 + 8 * a * t - 4 * a
    return 0.0


def _resize_mat(old, new):
    M = np.zeros((new, old), dtype=np.float64)
    for j in range(new):
        s = j * (old - 1) / (new - 1) if new > 1 else 0.0
        f = int(np.floor(s))
        for k in range(-1, 3):
            M[j, min(max(f + k, 0), old - 1)] += _cubic(s - (f + k))
    return M


def _dft_consts():
    k = np.arange(128)
    ang = 2 * np.pi * np.outer(k, k) / 128.0
    C = np.cos(ang) / np.sqrt(128.0)
    Sm = np.sin(ang) / np.sqrt(128.0)
    CWF = np.concatenate([C[:, 0:65], -Sm[:, 1:64]], axis=1)
    alpha = np.ones(65)
    alpha[1:64] = 2.0
    cwa = (alpha[:, None]
           * np.cos(2 * np.pi * np.outer(np.arange(65), k) / 128.0)
           / np.sqrt(128.0))
    swa = (2.0 * np.sin(2 * np.pi * np.outer(np.arange(1, 64), k) / 128.0)
           / np.sqrt(128.0))
    IWF = np.concatenate([cwa, -swa], axis=0)
    return (CWF.astype(np.float16), C.astype(np.float16),
            Sm.astype(np.float16), (-Sm).astype(np.float16),
            IWF.astype(np.float16))


def _host_routing_G(x, cw0, cw1, cw2, sp_w, bn_gamma, bn_beta, bn_mean,
                    bn_var, fc1, mlp_scale, mlp_bias, fc2, soa1_scale):
    x = np.asarray(x, np.float64)
    gctx = x.mean(axis=(1, 2))
    y = np.einsum('bhwc,sc->bhws', x, np.asarray(sp_w, np.float64))
    y = ((y - np.asarray(bn_mean, np.float64))
         / np.sqrt(np.asarray(bn_var, np.float64) + 1e-5)
         * np.asarray(bn_gamma, np.float64) + np.asarray(bn_beta, np.float64))
    sctx = np.maximum(y, 0.0).mean(axis=(1, 2))
    fused = np.concatenate([gctx, sctx], axis=1)
    hm = fused @ np.asarray(fc1, np.float64).T
    ms = float(np.asarray(mlp_scale).reshape(-1)[0])
    mb = float(np.asarray(mlp_bias).reshape(-1)[0])
    hmid = ms * np.maximum(hm, 0.0) ** 2 + mb
    logits = (hmid @ np.asarray(fc2, np.float64).T).reshape(B, NS, MED)
    e = np.exp(logits - logits.max(axis=1, keepdims=True))
    r = e / e.sum(axis=1, keepdims=True)
    filts = []
    for cw, (sh, sw) in zip((cw0, cw1, cw2), SCALE_HW):
        cw = np.asarray(cw, np.float64)
        Rh = _resize_mat(sh, FH).astype(np.float32)
        Rw = _resize_mat(sw, FWH).astype(np.float32)
        t = np.einsum('Vw,hwmc->hVmc', Rw, cw.astype(np.float32))
        t = np.einsum('Hh,hVmc->HVmc', Rh, t)
        filts.append((t[..., 0] + 1j * t[..., 1]).astype(np.complex64))
    filt = np.stack(filts)                                  # [3, kh, kw, m]
    sc = float(np.asarray(soa1_scale).reshape(-1)[0])
    G = np.einsum('skwm,bsm->bkmw', filt,
                  r.astype(np.complex64)) * sc              # [B, kh, m, kw]
    return r, G


# ---------------------------------------------------------------- device
def _build():
    nc = bacc.Bacc("TRN2", target_bir_lowering=False, debug=False,
                   num_devices=B)
    xt = nc.dram_tensor("xt", [DIM, S], F16, kind="ExternalInput").ap()
    gg = nc.dram_tensor("gg", [128, 2 * 192 * 65], F16, kind="ExternalInput").ap()
    w1x_d = nc.dram_tensor("w1x", [96, 192], F16, kind="ExternalInput").ap()
    cwf_d = nc.dram_tensor("cwf", [128, 128], F16, kind="ExternalInput").ap()
    cht_d = nc.dram_tensor("cht", [128, 128], F16, kind="ExternalInput").ap()
    sht_d = nc.dram_tensor("sht", [128, 128], F16, kind="ExternalInput").ap()
    nsht_d = nc.dram_tensor("nsht", [128, 128], F16, kind="ExternalInput").ap()
    iwf_d = nc.dram_tensor("iwf", [128, 128], F16, kind="ExternalInput").ap()
    w2a_d = nc.dram_tensor("w2a", [96, 96], F16, kind="ExternalInput").ap()
    w2b_d = nc.dram_tensor("w2b", [96, 96], F16, kind="ExternalInput").ap()
    out_d = nc.dram_tensor("out", [DIM, S], F16, kind="ExternalOutput").ap()

    with tile.TileContext(nc) as tc, ExitStack() as ctx:
        cpool = ctx.enter_context(tc.tile_pool(name="c", bufs=1))
        big = ctx.enter_context(tc.tile_pool(name="big", bufs=2))
        chp = ctx.enter_context(tc.tile_pool(name="ch", bufs=3))
        spc = ctx.enter_context(tc.tile_pool(name="sc", bufs=3))
        xpool = ctx.enter_context(tc.tile_pool(name="xp", bufs=2))
        fpool = ctx.enter_context(tc.tile_pool(name="fp", bufs=3))
        gpool = ctx.enter_context(tc.tile_pool(name="gp", bufs=4))
        tpool = ctx.enter_context(tc.tile_pool(name="tp", bufs=2))
        rpool = ctx.enter_context(tc.tile_pool(name="rp", bufs=3))
        opool = ctx.enter_context(tc.tile_pool(name="op", bufs=2))
        pp = ctx.enter_context(tc.tile_pool(name="ps", bufs=6, space="PSUM"))
        pp7 = ctx.enter_context(tc.tile_pool(name="p7", bufs=2, space="PSUM"))

        w1x = cpool.tile([96, 192], F16)
        cwf = cpool.tile([128, 128], F16, tag="cwf")
        cht = cpool.tile([128, 128], F16, tag="cht")
        sht = cpool.tile([128, 128], F16, tag="sht")
        nsht = cpool.tile([128, 128], F16, tag="nsht")
        iwf = cpool.tile([128, 128], F16, tag="iwf")
        w2a = cpool.tile([96, 96], F16, tag="w2a")
        w2b = cpool.tile([96, 96], F16, tag="w2b")
        nc.sync.dma_start(w1x[:], w1x_d[:])
        nc.sync.dma_start(cwf[:], cwf_d[:])
        nc.sync.dma_start(cht[:], cht_d[:])
        nc.sync.dma_start(sht[:], sht_d[:])
        nc.sync.dma_start(nsht[:], nsht_d[:])
        nc.sync.dma_start(iwf[:], iwf_d[:])
        nc.sync.dma_start(w2a[:], w2a_d[:])
        nc.sync.dma_start(w2b[:], w2b_d[:])

        BW = 24576          # big tile free width
        SCH = 24            # channels per S2 chunk (2 blocks)
        SCW = SCH * 128     # 3072
        TCH = 24            # channels per T1 chunk (2 blocks)
        TCW = TCH * 128     # 3072

        # ---------------- S1: pw1 + relu^2 -> x_pre [w, h*192+m]
        x_pre = big.tile([128, BW], F16, tag="big")
        XCH = 1024  # xt chunk cols (8 h)
        for ci in range(S // XCH):
            xc = xpool.tile([96, XCH], F16, tag="xt")
            nc.sync.dma_start(xc[:], xt[:, ci * XCH:(ci + 1) * XCH])
            for hh in range(0, 8, 2):
                h = ci * 8 + hh
                ps = pp.tile([128, 512], F32, tag="ps")
                for j in range(2):
                    nc.tensor.matmul(
                        ps[:, j * 192:(j + 1) * 192],
                        xc[:, (hh + j) * 128:(hh + j + 1) * 128],
                        w1x[:], start=True, stop=True)
                # relu^2: alternate (ACT relu, DVE sq) / (DVE max0, ACT sq)
                rt = rpool.tile([128, 384], F16, tag="rt")
                t4 = (h // 2) % 4
                if t4 % 2 == 0:
                    nc.scalar.activation(rt[:], ps[:, 0:384],
                                         mybir.ActivationFunctionType.Relu)
                    nc.vector.tensor_mul(x_pre[:, h * 192:(h + 2) * 192],
                                         rt[:], rt[:])
                elif t4 == 1:
                    nc.vector.tensor_scalar_max(rt[:], ps[:, 0:384], 0.0)
                    nc.gpsimd.tensor_mul(x_pre[:, h * 192:(h + 2) * 192],
                                         rt[:], rt[:])
                else:
                    nc.vector.tensor_scalar_max(rt[:], ps[:, 0:384], 0.0)
                    nc.scalar.square(x_pre[:, h * 192:(h + 2) * 192], rt[:])

        # ---------------- S2: rfft-W packed, chunked (48 ch) -> Xw chunks
        xp3 = x_pre[:].rearrange("p (h m) -> p m h", h=128, m=192)
        xw_chunks = {}

        def s2_chunk(c):
            xwt = chp.tile([128, SCW], F16, tag="xwt")
            for jj in range(SCH // 4):            # 6 matmuls of N=512
                j = c * (SCH // 4) + jj
                ps = pp.tile([128, 512], F32, tag="ps")
                nc.tensor.matmul(ps[:], cwf[:],
                                 xp3[:, j * 4:(j + 1) * 4, :],
                                 start=True, stop=True)
                if j % 3 != 0:
                    nc.scalar.copy(xwt[:, jj * 512:(jj + 1) * 512], ps[:])
                else:
                    nc.vector.tensor_copy(xwt[:, jj * 512:(jj + 1) * 512],
                                          ps[:])
            xw = chp.tile([128, SCW], F16, tag="xw")
            nc.sync.dma_start_transpose(
                xw[:].rearrange("p (j k) -> p j k", j=SCH, k=128), xwt[:])
            xw_chunks[c] = xw

        # ---------------- S3/M/S5 software-pipelined over 12-channel blocks
        Zf = big.tile([128, BW], F16, tag="big")
        zf3d = Zf[:].rearrange("p (j k) -> p j k", j=192, k=128)
        yts = {}
        zhs = {}

        def s3_and_m(blk):
            m0 = blk * MBLK
            xw3 = xw_chunks[blk // 2][:].rearrange(
                "p (m c) -> p m c", m=SCH, c=128)
            lm0 = (blk % 2) * MBLK
            gt = gpool.tile([128, 2 * FDB], F16, tag="g")
            nc.gpsimd.dma_start(gt[:], gg[:, blk * 2 * FDB:(blk + 1) * 2 * FDB])
            xf = fpool.tile([128, 2 * FDB], F16, tag="xf")
            for q in range(MBLK // GCH):
                g0 = lm0 + q * GCH
                NW = GCH * 65                     # 390
                re_in = xw3[:, g0:g0 + GCH, 0:65]
                im_in = xw3[:, g0:g0 + GCH, 65:128]
                psr = pp.tile([128, 512], F32, tag="ps")
                psi = pp.tile([128, 512], F32, tag="ps")
                psr3 = psr[:, 0:NW].rearrange("p (m c) -> p m c", m=GCH, c=65)
                psi3 = psi[:, 0:NW].rearrange("p (m c) -> p m c", m=GCH, c=65)
                nc.tensor.matmul(psr[:, 0:NW], cht[:], re_in,
                                 start=True, stop=False)
                nc.tensor.matmul(psr3[:, :, 1:64], sht[:], im_in,
                                 start=False, stop=True)
                nc.tensor.matmul(psi[:, 0:NW], nsht[:], re_in,
                                 start=True, stop=False)
                nc.tensor.matmul(psi3[:, :, 1:64], cht[:], im_in,
                                 start=False, stop=True)
                o = q * NW
                nc.scalar.copy(xf[:, o:o + NW], psr[:, 0:NW])
                nc.scalar.copy(xf[:, FDB + o:FDB + o + NW], psi[:, 0:NW])
            # M: Y = Xf * G (1 mul POOL, 3 muls + combines DVE)
            xfre, xfim = xf[:, 0:FDB], xf[:, FDB:2 * FDB]
            gre_t, gim_t = gt[:, 0:FDB], gt[:, FDB:2 * FDB]
            yt = fpool.tile([128, 2 * FDB], F16, tag="y")
            t1 = tpool.tile([128, FDB], F16, tag="mt")
            nc.vector.tensor_mul(yt[:, 0:FDB], xfre[:], gre_t[:])
            nc.vector.tensor_mul(t1[:], xfim[:], gim_t[:])
            nc.gpsimd.tensor_mul(yt[:, FDB:2 * FDB], xfim[:], gre_t[:])
            nc.vector.tensor_sub(yt[:, 0:FDB], yt[:, 0:FDB], t1[:])
            t2 = tpool.tile([128, FDB], F16, tag="mt")
            nc.vector.tensor_mul(t2[:], xfre[:], gim_t[:])
            nc.vector.tensor_add(yt[:, FDB:2 * FDB], yt[:, FDB:2 * FDB], t2[:])
            yts[blk] = yt

        def s5(blk):
            if blk % 2 == 0:
                zhc = chp.tile([128, TCW], F16, tag="zh")
                zhs[blk // 2] = zhc
            zh = zhs[blk // 2]
            lm0 = (blk % 2) * MBLK
            yt = yts.pop(blk)
            yre3 = yt[:, 0:FDB].rearrange("p (m c) -> p m c", m=MBLK, c=65)
            yim3 = yt[:, FDB:2 * FDB].rearrange("p (m c) -> p m c",
                                                m=MBLK, c=65)
            for q in range(MBLK // GCH):
                q0 = q * GCH
                NW = GCH * 65                     # 390
                NI = GCH * 63                     # 378
                re_in = yre3[:, q0:q0 + GCH, :]
                im_in = yim3[:, q0:q0 + GCH, :]
                psr = pp.tile([128, 512], F32, tag="ps")
                psi = pp.tile([128, 512], F32, tag="ps")
                nc.tensor.matmul(psr[:, 0:NW], cht[:], re_in,
                                 start=True, stop=False)
                nc.tensor.matmul(psr[:, 0:NW], nsht[:], im_in,
                                 start=False, stop=True)
                nc.tensor.matmul(psi[:, 0:NI], sht[:],
                                 yre3[:, q0:q0 + GCH, 1:64],
                                 start=True, stop=False)
                nc.tensor.matmul(psi[:, 0:NI], cht[:],
                                 yim3[:, q0:q0 + GCH, 1:64],
                                 start=False, stop=True)
                zh3 = zh[:, (lm0 + q0) * 128:(lm0 + q0 + GCH) * 128].rearrange(
                    "p (m c) -> p m c", m=GCH, c=128)
                psr3 = psr[:, 0:NW].rearrange("p (m c) -> p m c", m=GCH, c=65)
                psi3 = psi[:, 0:NI].rearrange("p (m c) -> p m c", m=GCH, c=63)
                nc.scalar.copy(zh3[:, :, 0:65], psr3[:])
                nc.scalar.copy(zh3[:, :, 65:128], psi3[:])
            if blk % 2 == 1:        # 24 channels done -> T1 chunk
                c = blk // 2
                nc.sync.dma_start_transpose(
                    zf3d[:, c * TCH:(c + 1) * TCH, :], zhs.pop(c)[:])

        s2_chunk(0)
        s2_chunk(1)
        for blk in range(NBLK):
            if blk % 2 == 0 and blk // 2 + 2 < MED // SCH:
                s2_chunk(blk // 2 + 2)
            s3_and_m(blk)
            if blk >= 2:
                s5(blk - 2)
        s5(NBLK - 2)
        s5(NBLK - 1)

        # ---------------- S6 (stationary-data irfft-W, no transpose) + S7
        # per 4-h group: lhsT = Zf[:, m*128+h] slices -> psum [m-chunk, 4*128]
        def s6_s7(hb):
            h4 = hb * 4
            dd = []
            for base in (0, 12288):              # m 0..95 | m 96..191
                ps = pp.tile([128, 512], F32, tag="ps")
                for k in range(4):
                    h = h4 + k
                    nc.tensor.matmul(
                        ps[0:96, k * 128:(k + 1) * 128],
                        Zf[:, base + h:base + h + 95 * 128 + 1:128],
                        iwf[:], start=True, stop=True)
                d = spc.tile([96, 512], F16, tag="dA" if base == 0 else "dB")
                if hb % 2 == 0:
                    nc.scalar.copy(d[:], ps[0:96, :])
                else:
                    nc.vector.tensor_copy(d[:], ps[0:96, :])
                dd.append(d)
            ps7 = pp7.tile([128, 512], F32, tag="p7")
            nc.tensor.matmul(ps7[0:96, :], w2a[:], dd[0][:],
                             start=True, stop=False)
            nc.tensor.matmul(ps7[0:96, :], w2b[:], dd[1][:],
                             start=False, stop=True)
            if hb % 4 == 0:
                obt = opool.tile([96, 2048], F16, tag="ob")
                obs[0] = obt
            ob = obs[0]
            sl = slice((hb % 4) * 512, (hb % 4 + 1) * 512)
            if hb % 2 == 0:
                nc.vector.tensor_copy(ob[:, sl], ps7[0:96, :])
            else:
                nc.scalar.copy(ob[:, sl], ps7[0:96, :])
            if hb % 4 == 3:
                nc.scalar.dma_start(
                    out_d[:, (h4 - 12) * 128:(h4 + 4) * 128], ob[:])

        obs = [None]
        for hb in range(32):
            s6_s7(hb)
    nc.finalize()
    return nc


_CACHE = {}
LAST_EXEC_NS = 0


def _consts_f16(w1, w2):
    CWF, CHT, SHT, NSHT, IWF = _dft_consts()
    return dict(
        w1x=np.ascontiguousarray(np.asarray(w1, np.float32).T).astype(np.float16),
        cwf=CWF, cht=CHT, sht=SHT, nsht=NSHT, iwf=IWF,
        w2a=np.ascontiguousarray(np.asarray(w2, np.float32).T[0:96]).astype(np.float16),
        w2b=np.ascontiguousarray(np.asarray(w2, np.float32).T[96:192]).astype(np.float16),
    )


def kernel(x, w1, soa1_scale, soa1_bias, cw0, cw1, cw2, sp_w,
           bn_gamma, bn_beta, bn_mean, bn_var,
           fc1, mlp_scale, mlp_bias, fc2, w2):
    global LAST_EXEC_NS
    x = np.asarray(x, np.float32)
    r, G = _host_routing_G(x, cw0, cw1, cw2, sp_w, bn_gamma, bn_beta,
                           bn_mean, bn_var, fc1, mlp_scale, mlp_bias, fc2,
                           soa1_scale)
    consts = _consts_f16(w1, w2)
    in_maps = []
    for b in range(B):
        xT = np.ascontiguousarray(x[b].reshape(S, DIM).T).astype(np.float16)
        m = dict(consts)
        m["xt"] = xT
        gr = G[b].real.reshape(128, NBLK, FDB)
        gi = G[b].imag.reshape(128, NBLK, FDB)
        ggb = np.empty((128, NBLK, 2 * FDB), np.float16)
        ggb[:, :, 0:FDB] = gr
        ggb[:, :, FDB:2 * FDB] = gi
        m["gg"] = np.ascontiguousarray(ggb.reshape(128, 2 * 192 * 65))
        in_maps.append(m)

    if "nc" not in _CACHE:
        _CACHE["nc"] = _build()
    nc = _CACHE["nc"]
    res = run_bass_kernel_spmd(nc, in_maps, list(range(B)))
    if os.environ.get("KERNEL_TRACE_EXEC") and "tl" not in _CACHE:
        from concourse.timeline_sim import TimelineSim
        _CACHE["tl"] = TimelineSim(nc, trace=False).simulate()
    if _CACHE.get("tl"):
        LAST_EXEC_NS = int(_CACHE["tl"])

    outs = np.empty((B, H, W, DIM), np.float32)
    for b in range(B):
        o = res.results[b]["out"].astype(np.float32)      # [c, hw]
        outs[b] = o.T.reshape(H, W, DIM)

    bias = float(np.asarray(soa1_bias).reshape(-1)[0])
    if bias != 0.0:
        comb00 = G[:, 0, :, 0]                            # [B, m]
        corr = np.real(comb00).astype(np.float64) @ np.asarray(w2, np.float64).T
        outs = outs + bias * corr[:, None, None, :].astype(np.float32)
    return outs
